# revision 20
# baseline (speedup 1.0000x reference)
"""Trainium2 Bass kernel for a dense transformer block (B=2, S=2048, E=768, H=12).

Sharding: 8 cores = 2 batch groups x 4 ranks. Head-parallel attention:
core (g, r) owns heads [3r, 3r+3) of batch element g and token rows
[512r, 512r+512) for everything token-local (residuals, LN2, FFN, output).

The host replicates x^T (bf16) across each batch group, so LN1 stats and
Q/K/V projections for the core's own heads over the FULL sequence start
immediately with no collective. After attention, each core holds ctx for
its 3 heads over all 2048 tokens; a per-head 8-core AllToAll sends each
rank the ctx slice for its own 512 tokens. The receive frame interleaves
both batch groups; the output projection contracts over the full 1536-row
frame with a host-permuted wo whose cross-group rows are zeroed, keeping
the device program SPMD-uniform. FFN is token-parallel with full streamed
weights. All matmul operands are bf16 (fp32 PSUM accumulation); softmax
skips max-subtraction and gets the denominator via a ones-augmented V
column.
"""

import numpy as np

B, S, E, H, D = 2, 2048, 768, 12, 64
F = 4 * E
NCORES = 8
TPG = 4                 # ranks per batch group
T = S // TPG            # 512 own tokens
HPC = H // TPG          # 3 heads per core
HD = HPC * D            # 192 own head dims
P = 128
EC = E // P             # 6 embed chunks
FC = F // P             # 24 ffn-hidden chunks
TC = T // P             # 4 own token chunks
KC = S // P             # 16 key chunks (full seq)
QB = 2                  # query blocks of 1024
QW = S // QB            # 1024
EPS = 1e-5
SCALE = 1.0 / float(np.sqrt(E))

_CACHE = {}


def _build_nc():
    import concourse.bass as bass
    import concourse.mybir as mybir
    import concourse.tile as tile
    from concourse import bacc
    from concourse.masks import make_identity

    dt = mybir.dt
    f32 = dt.float32
    bf16 = dt.bfloat16
    Alu = mybir.AluOpType
    Act = mybir.ActivationFunctionType
    Axis = mybir.AxisListType

    nc = bacc.Bacc(
        "TRN2",
        target_bir_lowering=False,
        debug=False,
        enable_asserts=False,
        num_devices=NCORES,
    )

    xT_in = nc.dram_tensor("xT", [E, S], bf16, kind="ExternalInput")
    xo_in = nc.dram_tensor("x_own", [T, E], f32, kind="ExternalInput")
    wq_in = nc.dram_tensor("wq", [E, HD], bf16, kind="ExternalInput")
    wk_in = nc.dram_tensor("wk", [E, HD], bf16, kind="ExternalInput")
    wv_in = nc.dram_tensor("wv", [E, HD], bf16, kind="ExternalInput")
    bq_in = nc.dram_tensor("bq", [2 * P], f32, kind="ExternalInput")
    bk_in = nc.dram_tensor("bk", [2 * P], f32, kind="ExternalInput")
    bv_in = nc.dram_tensor("bv", [HD], f32, kind="ExternalInput")
    wop_in = nc.dram_tensor("wop", [NCORES * HD, E], bf16, kind="ExternalInput")
    bo_in = nc.dram_tensor("bo", [E], f32, kind="ExternalInput")
    ln1g_in = nc.dram_tensor("ln1_g", [E], f32, kind="ExternalInput")
    ln1b_in = nc.dram_tensor("ln1_b", [E], f32, kind="ExternalInput")
    ln2g_in = nc.dram_tensor("ln2_g", [E], f32, kind="ExternalInput")
    ln2b_in = nc.dram_tensor("ln2_b", [E], f32, kind="ExternalInput")
    w1_in = nc.dram_tensor("w1", [E, F], bf16, kind="ExternalInput")
    b1_in = nc.dram_tensor("b1", [F], f32, kind="ExternalInput")
    w2_in = nc.dram_tensor("w2", [F, E], bf16, kind="ExternalInput")
    b2_in = nc.dram_tensor("b2", [E], f32, kind="ExternalInput")
    out_dram = nc.dram_tensor("out", [T, E], f32, kind="ExternalOutput")
    import os as _os
    DBG = bool(_os.environ.get("KBUILD_DEBUG"))
    if DBG:
        dbg_rs = nc.dram_tensor("dbg_rs", [P, S], bf16, kind="ExternalOutput")
        dbg_mu = nc.dram_tensor("dbg_mu", [P, S], bf16, kind="ExternalOutput")
        dbg_xh = nc.dram_tensor("dbg_xh", [P, S], bf16, kind="ExternalOutput")
        dbg_kt = nc.dram_tensor("dbg_kt", [P, S], bf16, kind="ExternalOutput")
        dbg_v3 = nc.dram_tensor("dbg_v3", [P, KC * HPC * (D + 1)], bf16, kind="ExternalOutput")
        dbg_ctx = nc.dram_tensor("dbg_ctx", [HPC, 64, S], bf16, kind="ExternalOutput")
        dbg_a2i = nc.dram_tensor("dbg_a2i", [NCORES, D, T], bf16, kind="ExternalOutput")
        dbg_a2o = nc.dram_tensor("dbg_a2o", [NCORES, D, T], bf16, kind="ExternalOutput")
        dbg_ca = nc.dram_tensor("dbg_ca", [P, 2 * EC * T], bf16, kind="ExternalOutput")
        dbg_y = nc.dram_tensor("dbg_y", [P, TC * E], f32, kind="ExternalOutput")
        dbg_y2t = nc.dram_tensor("dbg_y2t", [P, EC * T], bf16, kind="ExternalOutput")
        dbg_h = nc.dram_tensor("dbg_h", [P, FC * T], bf16, kind="ExternalOutput")

    # per-own-head AllToAll bounce buffers
    a2a_in = nc.dram_tensor("a2a_in", [HPC, NCORES, D, T], bf16)
    a2a_out = nc.dram_tensor("a2a_out", [HPC, NCORES, D, T], bf16)
    a2a_groups = [list(range(NCORES))]

    with tile.TileContext(nc) as tc:
        const_pool = tc.alloc_tile_pool(name="const", bufs=1)
        acts = tc.alloc_tile_pool(name="acts", bufs=1)
        stream = tc.alloc_tile_pool(name="stream", bufs=1)

        # ---------------- constants ----------------
        ident = const_pool.tile([P, P], bf16)
        make_identity(nc, ident)
        ones_col = const_pool.tile([P, 1], bf16)
        nc.vector.memset(ones_col, 1.0)
        ones64 = const_pool.tile([1, 64], bf16)
        nc.vector.memset(ones64, 1.0)
        eps_col = const_pool.tile([1, 1], f32)
        nc.vector.memset(eps_col, EPS)
        eps_col2 = const_pool.tile([P, 1], f32)
        nc.vector.memset(eps_col2, EPS)

        ln1g_col = const_pool.tile([P, EC], f32)
        nc.sync.dma_start(ln1g_col, ln1g_in.rearrange("(c p) -> p c", p=P))
        ln1b_col = const_pool.tile([P, EC], f32)
        nc.sync.dma_start(ln1b_col, ln1b_in.rearrange("(c p) -> p c", p=P))
        bqc = const_pool.tile([P, 2], f32)
        nc.sync.dma_start(bqc, bq_in.rearrange("(c p) -> p c", p=P))
        bkc = const_pool.tile([P, 2], f32)
        nc.sync.dma_start(bkc, bk_in.rearrange("(c p) -> p c", p=P))
        b1_col = const_pool.tile([P, FC], f32)
        nc.sync.dma_start(b1_col, b1_in.rearrange("(c p) -> p c", p=P))

        # free-axis rows replicated across partitions
        reps = {}
        for name, t_in, width in [
            ("bv", bv_in, HD), ("bo", bo_in, E), ("b2", b2_in, E),
            ("ln2_g", ln2g_in, E), ("ln2_b", ln2b_in, E),
        ]:
            row = const_pool.tile([1, width], f32, name=f"{name}_row")
            nc.sync.dma_start(row, t_in[None, :])
            rep = const_pool.tile([P, width], f32, name=f"{name}_rep")
            nc.gpsimd.partition_broadcast(rep, row)
            reps[name] = rep

        # ================ phase 1: stats, x-hat, QKV ================
        ph1_sb = tc.alloc_tile_pool(name="ph1_sb", bufs=1)
        ph1a_ps = tc.alloc_tile_pool(name="ph1a_ps", bufs=1, space="PSUM")

        xt = ph1_sb.tile([P, EC, S], bf16)
        xt_v = xT_in.rearrange("(c p) t -> p c t", p=P)
        for ec in range(EC):
            nc.sync.dma_start(xt[:, ec, :], xt_v[:, ec, :])
        xo = acts.tile([P, TC, E], f32)
        nc.sync.dma_start(xo, xo_in.rearrange("(c p) e -> p c e", p=P))

        wq_sb = ph1_sb.tile([P, EC, HD], bf16)
        nc.sync.dma_start(wq_sb, wq_in.rearrange("(c p) d -> p c d", p=P))
        wk_sb = ph1_sb.tile([P, EC, HD], bf16)
        nc.sync.dma_start(wk_sb, wk_in.rearrange("(c p) d -> p c d", p=P))
        wv_sb = ph1_sb.tile([P, EC, HD], bf16)
        nc.sync.dma_start(wv_sb, wv_in.rearrange("(c p) d -> p c d", p=P))

        # LN1 stats for all 2048 tokens: col-sums of x and x^2 via PE
        st_s = [
            ph1a_ps.tile([1, 512], f32, tag=f"sts{qb}", bufs=1, name=f"sts{qb}")
            for qb in range(4)
        ]
        st_q = [
            ph1a_ps.tile([1, 512], f32, tag=f"stq{qb}", bufs=1, name=f"stq{qb}")
            for qb in range(4)
        ]
        for ec in range(EC):
            sq = stream.tile([P, S], bf16, tag="sq", bufs=2, name="sq")
            nc.scalar.activation(sq, xt[:, ec, :], Act.Square)
            for qb in range(4):
                sl = slice(qb * 512, (qb + 1) * 512)
                nc.tensor.matmul(
                    st_s[qb], ones_col, xt[:, ec, sl],
                    start=(ec == 0), stop=(ec == EC - 1),
                )
                nc.tensor.matmul(
                    st_q[qb], ones_col, sq[:, sl],
                    start=(ec == 0), stop=(ec == EC - 1),
                )

        rs_b = ph1_sb.tile([P, S], bf16)
        murs_b = ph1_sb.tile([P, S], bf16)
        for qb in range(4):
            sl = slice(qb * 512, (qb + 1) * 512)
            mean = ph1_sb.tile([1, 512], f32, name=f"mean{qb}")
            nc.vector.tensor_scalar(mean, st_s[qb], 1.0 / E, None, Alu.mult)
            var = ph1_sb.tile([1, 512], f32, name=f"var{qb}")
            nc.vector.tensor_scalar(var, st_q[qb], 1.0 / E, None, Alu.mult)
            msq = ph1_sb.tile([1, 512], f32, name=f"msq{qb}")
            nc.vector.tensor_tensor(msq, mean, mean, Alu.mult)
            nc.vector.tensor_tensor(var, var, msq, Alu.subtract)
            lnv = ph1_sb.tile([1, 512], f32, name=f"lnv{qb}")
            nc.scalar.activation(lnv, var, Act.Ln, bias=eps_col)
            rsq = ph1_sb.tile([1, 512], f32, name=f"rsq{qb}")
            nc.scalar.activation(rsq, lnv, Act.Exp, scale=-0.5)
            rs_bf = ph1_sb.tile([1, 512], bf16, name=f"rsbf{qb}")
            nc.vector.tensor_copy(rs_bf, rsq)
            murs_bf = ph1_sb.tile([1, 512], bf16, name=f"mursbf{qb}")
            nc.vector.tensor_tensor(murs_bf, mean, rsq, Alu.mult)
            nc.gpsimd.partition_broadcast(rs_b[:, sl], rs_bf)
            nc.gpsimd.partition_broadcast(murs_b[:, sl], murs_bf)

        # x-hat^T = ((x*rs) - mu*rs) * g + b   (bf16, in-place over xt)
        xhat = xt
        for ec in range(EC):
            t1 = stream.tile([P, S], bf16, tag="xh1", bufs=2, name="xh1")
            nc.vector.tensor_tensor(t1, xt[:, ec, :], rs_b, Alu.mult)
            nc.vector.tensor_tensor(t1, t1, murs_b, Alu.subtract)
            nc.vector.tensor_scalar(
                xhat[:, ec, :], t1,
                ln1g_col[:, ec, None], ln1b_col[:, ec, None],
                Alu.mult, Alu.add,
            )

        ph1a_ps.release()
        ph1b_ps = tc.alloc_tile_pool(name="ph1b_ps", bufs=1, space="PSUM")

        # K^T and Q^T for own heads over all tokens: [HD rows, S]
        kT_a = acts.tile([P, S], bf16)
        kT_b = acts.tile([64, S], bf16)
        qT_a = acts.tile([P, S], bf16)
        qT_b = acts.tile([64, S], bf16)
        for (w_sb, bc_col, dst_a, dst_b) in (
            (wk_sb, bkc, kT_a, kT_b),
            (wq_sb, bqc, qT_a, qT_b),
        ):
            for qb in range(4):
                sl = slice(qb * 512, (qb + 1) * 512)
                psa = ph1b_ps.tile([P, 512], f32, tag="proj", bufs=2, name="proj")
                psb = ph1b_ps.tile([64, 512], f32, tag="projB", bufs=2, name="projB")
                for ec in range(EC):
                    nc.tensor.matmul(
                        psa, w_sb[:, ec, 0:P], xhat[:, ec, sl],
                        start=(ec == 0), stop=(ec == EC - 1),
                    )
                    nc.tensor.matmul(
                        psb, w_sb[:, ec, P:HD], xhat[:, ec, sl],
                        start=(ec == 0), stop=(ec == EC - 1),
                    )
                nc.vector.tensor_scalar(
                    dst_a[:, sl], psa, bc_col[:, 0, None], None, Alu.add
                )
                nc.vector.tensor_scalar(
                    dst_b[:, sl], psb, bc_col[0:64, 1, None], None, Alu.add
                )

        # V natural (per key chunk), ones-augmented: [128k, KC, HPC, D+1]
        v3 = acts.tile([P, KC, HPC, D + 1], bf16)
        for kc in range(KC):
            for i in range(HPC):
                nc.vector.memset(v3[:, kc, i, D, None], 1.0)
        for kc in range(KC):
            vp = ph1b_ps.tile([P, HD], f32, tag="vp", bufs=2, name="vp")
            tsl = slice(kc * P, (kc + 1) * P)
            for ec in range(EC):
                nc.tensor.matmul(
                    vp, xhat[:, ec, tsl], wv_sb[:, ec, :],
                    start=(ec == 0), stop=(ec == EC - 1),
                )
            for i in range(HPC):
                nc.vector.tensor_tensor(
                    v3[:, kc, i, 0:D], vp[:, i * D:(i + 1) * D],
                    reps["bv"][:, i * D:(i + 1) * D], Alu.add,
                )

        if DBG:
            nc.sync.dma_start(dbg_rs[:, :], rs_b)
            nc.sync.dma_start(dbg_mu[:, :], murs_b)
            nc.sync.dma_start(dbg_xh[:, :], xhat[:, 0, :])
            nc.sync.dma_start(dbg_kt[:, :], kT_a)
            nc.sync.dma_start(dbg_v3[:, :], v3.rearrange("p a b c -> p (a b c)"))

        ph1_sb.release()
        ph1b_ps.release()

        # ================ phase 2: attention (3 own heads) ================
        att_sb = tc.alloc_tile_pool(name="att_sb", bufs=1)
        att_ps = tc.alloc_tile_pool(name="att_ps", bufs=1, space="PSUM")

        # prefetch heavy phase-3 weights early (overlaps attention)
        wop_sb = att_sb.tile([P, 2 * EC, E], bf16)
        nc.sync.dma_start(wop_sb, wop_in.rearrange("(c p) o -> p c o", p=P))

        for i in range(HPC):
            if i == 0:
                krows, qrows = kT_a[0:64], qT_a[0:64]
            elif i == 1:
                krows, qrows = kT_a[64:128], qT_a[64:128]
            else:
                krows, qrows = kT_b[0:64], qT_b[0:64]
            ctxT = att_sb.tile([64, S], bf16, tag="ctxT", bufs=2, name="ctxT")
            for qb in range(QB):
                ctx_ps = att_ps.tile([D + 1, QW], f32, tag="ctx", bufs=2, name="ctx")
                exps = [None] * KC

                def emit_av(kc):
                    for h2 in range(2):
                        nc.tensor.matmul(
                            ctx_ps[:, h2 * 512:(h2 + 1) * 512],
                            v3[:, kc, i, :],
                            exps[kc][:, h2 * 512:(h2 + 1) * 512],
                            start=(kc == 0), stop=(kc == KC - 1),
                        )

                for kc in range(KC):
                    s_ps = att_ps.tile([P, QW], f32, tag="sps", bufs=2, name="sps")
                    for h2 in range(2):
                        nc.tensor.matmul(
                            s_ps[:, h2 * 512:(h2 + 1) * 512],
                            krows[:, kc * P:(kc + 1) * P],
                            qrows[:, qb * QW + h2 * 512: qb * QW + (h2 + 1) * 512],
                            start=True, stop=True,
                        )
                    exps[kc] = att_sb.tile([P, QW], bf16, tag="exp", bufs=4, name="exp")
                    nc.scalar.activation(exps[kc], s_ps, Act.Exp, scale=SCALE)
                    if kc >= 2:
                        emit_av(kc - 2)
                emit_av(KC - 2)
                emit_av(KC - 1)
                den = att_sb.tile([1, QW], f32, tag="den", bufs=1, name="den")
                nc.vector.tensor_copy(den, ctx_ps[D:D + 1, :])
                den_f = att_sb.tile([1, QW], f32, tag="denf", bufs=1, name="denf")
                nc.vector.reciprocal_approx_fast(den_f, den)
                den_b = att_sb.tile([1, QW], bf16, tag="denb", bufs=1, name="denb")
                nc.vector.tensor_copy(den_b, den_f)
                bc_ps = att_ps.tile([P, QW], f32, tag="sps", bufs=2, name="bcps")
                for h2 in range(2):
                    nc.tensor.matmul(
                        bc_ps[0:64, h2 * 512:(h2 + 1) * 512],
                        ones64, den_b[:, h2 * 512:(h2 + 1) * 512],
                        start=True, stop=True,
                    )
                bc_sb = att_sb.tile([64, QW], bf16, tag="bcsb", bufs=1, name="bcsb")
                nc.vector.tensor_copy(bc_sb, bc_ps[0:64, :])
                nc.vector.tensor_tensor(
                    ctxT[:, qb * QW:(qb + 1) * QW], ctx_ps[0:64, :], bc_sb, Alu.mult
                )
            if DBG:
                nc.sync.dma_start(dbg_ctx[i], ctxT)
            for j in range(NCORES):
                r = j % TPG
                nc.sync.dma_start(a2a_in[i, j], ctxT[:, r * T:(r + 1) * T])
            nc.gpsimd.collective_compute(
                "AllToAll", mybir.AluOpType.bypass,
                replica_groups=a2a_groups,
                ins=[a2a_in[i]],
                outs=[a2a_out[i]],
            )

        # ================ phase 3: assemble ctx, out-proj, residual =======
        # frame row j*HD + i*D + d  <->  a2a_out[i, j, d, :]
        ctx_all = att_sb.tile([P, 2 * EC, T], bf16)
        for j in range(NCORES):
            for i in range(HPC):
                row = j * HD + i * D
                cc, po = row // P, row % P
                nc.sync.dma_start(ctx_all[po:po + D, cc, :], a2a_out[i, j])

        if DBG:
            nc.sync.dma_start(dbg_ca[:, :], ctx_all.rearrange("p a b -> p (a b)"))
            for j in range(NCORES):
                bnc_i = stream.tile([64, T], bf16, tag="bnci", bufs=1, name="bnci")
                nc.sync.dma_start(bnc_i, a2a_in[0, j])
                nc.sync.dma_start(dbg_a2i[j], bnc_i)
                bnc_o = stream.tile([64, T], bf16, tag="bnco", bufs=1, name="bnco")
                nc.sync.dma_start(bnc_o, a2a_out[0, j])
                nc.sync.dma_start(dbg_a2o[j], bnc_o)
        att_ps.release()
        ph3_ps = tc.alloc_tile_pool(name="ph3_ps", bufs=1, space="PSUM")

        y_sb = acts.tile([P, TC, E], f32)
        for c in range(TC):
            tsl = slice(c * P, (c + 1) * P)
            for off, wdt in ((0, 512), (512, 256)):
                ps = ph3_ps.tile([P, 512], f32, tag="zo", bufs=2, name="zo")[:, :wdt]
                cc_order = [0, 3, 6, 9, 1, 2, 4, 5, 7, 8, 10, 11]
                for n_cc, cc in enumerate(cc_order):
                    nc.tensor.matmul(
                        ps, ctx_all[:, cc, tsl], wop_sb[:, cc, off:off + wdt],
                        start=(n_cc == 0), stop=(n_cc == 2 * EC - 1),
                    )
                osl = slice(off, off + wdt)
                nc.vector.tensor_tensor(
                    y_sb[:, c, osl], ps, reps["bo"][:, osl], Alu.add
                )
                nc.vector.tensor_tensor(
                    y_sb[:, c, osl], y_sb[:, c, osl], xo[:, c, osl], Alu.add
                )
        att_sb.release()

        if DBG:
            nc.sync.dma_start(dbg_y[:, :], y_sb.rearrange("p a b -> p (a b)"))
        # ================ phase 4: LN2 + transpose ================
        ffn_sb = tc.alloc_tile_pool(name="ffn_sb", bufs=1)
        stats2 = ffn_sb.tile([P, TC, 4], f32)
        s2 = stats2[:, :, 0]
        ss2 = stats2[:, :, 1]
        m2 = stats2[:, :, 2]
        r2 = stats2[:, :, 3]
        for c in range(TC):
            sq2 = stream.tile([P, E], f32, tag="sq2", bufs=2, name="sq2")
            nc.vector.tensor_reduce(s2[:, c, None], y_sb[:, c, :], Axis.X, Alu.add)
            nc.vector.tensor_tensor(sq2, y_sb[:, c, :], y_sb[:, c, :], Alu.mult)
            nc.vector.tensor_reduce(ss2[:, c, None], sq2, Axis.X, Alu.add)
        nc.vector.tensor_scalar(m2[:, :, None], s2[:, :, None], 1.0 / E, None, Alu.mult)
        var2 = ffn_sb.tile([P, TC], f32)
        nc.vector.tensor_scalar(var2[:, :, None], ss2[:, :, None], 1.0 / E, None, Alu.mult)
        msq2 = ffn_sb.tile([P, TC], f32)
        nc.vector.tensor_tensor(msq2[:, :, None], m2[:, :, None], m2[:, :, None], Alu.mult)
        nc.vector.tensor_tensor(var2[:, :, None], var2[:, :, None], msq2[:, :, None], Alu.subtract)
        lnv2 = ffn_sb.tile([P, TC], f32)
        nc.scalar.activation(lnv2[:, :, None], var2[:, :, None], Act.Ln, bias=eps_col2)
        nc.scalar.activation(r2[:, :, None], lnv2[:, :, None], Act.Exp, scale=-0.5)

        y2 = ffn_sb.tile([P, TC, E], bf16)
        for c in range(TC):
            nc.vector.tensor_scalar(
                y2[:, c, :], y_sb[:, c, :],
                m2[:, c, None], r2[:, c, None],
                Alu.subtract, Alu.mult,
            )
            nc.vector.tensor_tensor(y2[:, c, :], y2[:, c, :], reps["ln2_g"], Alu.mult)
            nc.vector.tensor_tensor(y2[:, c, :], y2[:, c, :], reps["ln2_b"], Alu.add)

        y2T = ffn_sb.tile([P, EC, T], bf16)
        for ec in range(EC):
            for c in range(TC):
                tps = ph3_ps.tile([P, P], bf16, tag="tp", bufs=2, name="tp")
                nc.tensor.transpose(tps, y2[:, c, ec * P:(ec + 1) * P], ident)
                nc.vector.tensor_copy(y2T[:, ec, c * P:(c + 1) * P], tps)
        ph3_ps.release()

        if DBG:
            nc.sync.dma_start(dbg_y2t[:, :], y2T.rearrange("p a b -> p (a b)"))
        # ================ phase 5: FFN ================
        ffn_ps = tc.alloc_tile_pool(name="ffn_ps", bufs=1, space="PSUM")
        hT = ffn_sb.tile([P, FC, T], bf16)
        w2_sb = ffn_sb.tile([P, FC, E], bf16)
        nc.sync.dma_start(w2_sb, w2_in.rearrange("(c p) o -> p c o", p=P))
        for fc in range(FC):
            w1b = ffn_sb.tile([P, EC, P], bf16, tag="w1b", bufs=3, name="w1b")
            nc.sync.dma_start(
                w1b, w1_in[:, fc * P:(fc + 1) * P].rearrange("(c p) h -> p c h", p=P)
            )
            hps = ffn_ps.tile([P, T], f32, tag="h", bufs=3, name="h")
            for ec in range(EC):
                nc.tensor.matmul(
                    hps, w1b[:, ec, :], y2T[:, ec, :],
                    start=(ec == 0), stop=(ec == EC - 1),
                )
            nc.scalar.activation(hT[:, fc, :], hps, Act.Gelu, bias=b1_col[:, fc, None])

        if DBG:
            nc.sync.dma_start(dbg_h[:, :], hT.rearrange("p a b -> p (a b)"))
        for c in range(TC):
            tsl = slice(c * P, (c + 1) * P)
            za = ffn_ps.tile([P, 512], f32, tag="zf1", bufs=2, name="zf1")
            zb = ffn_ps.tile([P, 256], f32, tag="zf2", bufs=2, name="zf2")
            for fc in range(FC):
                nc.tensor.matmul(
                    za, hT[:, fc, tsl], w2_sb[:, fc, 0:512],
                    start=(fc == 0), stop=(fc == FC - 1),
                )
                nc.tensor.matmul(
                    zb, hT[:, fc, tsl], w2_sb[:, fc, 512:768],
                    start=(fc == 0), stop=(fc == FC - 1),
                )
            o_sb = stream.tile([P, E], f32, tag="o", bufs=2, name="o")
            nc.vector.tensor_tensor(o_sb[:, 0:512], za, y_sb[:, c, 0:512], Alu.add)
            nc.vector.tensor_tensor(o_sb[:, 512:768], zb, y_sb[:, c, 512:768], Alu.add)
            nc.vector.tensor_tensor(o_sb, o_sb, reps["b2"], Alu.add)
            nc.sync.dma_start(out_dram[c * P:(c + 1) * P, :], o_sb)

        ffn_ps.release()
        ffn_sb.release()
        stream.release()
        acts.release()
        const_pool.release()

    nc.finalize()
    return nc


def _get_nc():
    if "nc" not in _CACHE:
        _CACHE["nc"] = _build_nc()
    return _CACHE["nc"]


def _shard_inputs(inputs):
    import ml_dtypes

    bf16 = ml_dtypes.bfloat16
    x = np.asarray(inputs["x"], dtype=np.float32)
    f = {k: np.asarray(v, dtype=np.float32) for k, v in inputs.items() if k != "x"}

    xT = [np.ascontiguousarray(x[g].T).astype(bf16) for g in range(B)]
    wo = f["wo"]

    in_maps = []
    for c in range(NCORES):
        g, r = c // TPG, c % TPG
        hsl = slice(HD * r, HD * r + HD)

        wop = np.zeros((NCORES * HD, E), np.float32)
        for j in range(NCORES):
            if j // TPG == g:
                wop[j * HD:(j + 1) * HD] = wo[HD * (j % TPG): HD * (j % TPG) + HD]

        def pad(b):
            v = np.zeros(2 * P, np.float32)
            v[:HD] = b
            return v

        m = {
            "xT": xT[g],
            "x_own": np.ascontiguousarray(x[g, r * T:(r + 1) * T]),
            "wq": np.ascontiguousarray(f["wq"][:, hsl]).astype(bf16),
            "wk": np.ascontiguousarray(f["wk"][:, hsl]).astype(bf16),
            "wv": np.ascontiguousarray(f["wv"][:, hsl]).astype(bf16),
            "bq": pad(f["bq"][hsl]),
            "bk": pad(f["bk"][hsl]),
            "bv": np.ascontiguousarray(f["bv"][hsl]),
            "wop": wop.astype(bf16),
            "bo": f["bo"],
            "ln1_g": f["ln1_g"], "ln1_b": f["ln1_b"],
            "ln2_g": f["ln2_g"], "ln2_b": f["ln2_b"],
            "w1": f["w1"].astype(bf16), "b1": f["b1"],
            "w2": f["w2"].astype(bf16), "b2": f["b2"],
        }
        in_maps.append(m)
    return in_maps


def kernel(**inputs):
    from concourse.bass_utils import run_bass_kernel_spmd

    nc = _get_nc()
    in_maps = _shard_inputs(inputs)
    res = run_bass_kernel_spmd(nc, in_maps, core_ids=list(range(NCORES)))
    _CACHE["last_results"] = res
    out = np.empty((B, S, E), np.float32)
    for c in range(NCORES):
        g, r = c // TPG, c % TPG
        out[g, r * T:(r + 1) * T, :] = res.results[c]["out"]
    return out


# revision 23
# speedup vs baseline: 1.1088x; 1.1088x over previous
"""Trainium2 Bass kernel for a dense transformer block (B=2, S=2048, E=768, H=12).

Sharding: 8 cores = 2 batch groups x 4 ranks. Head-parallel attention:
core (g, r) owns heads [3r, 3r+3) of batch element g and token rows
[512r, 512r+512) for everything token-local (residuals, LN2, FFN, output).

The host replicates x^T (bf16) across each batch group, so LN1 stats and
Q/K/V projections for the core's own heads over the FULL sequence start
immediately with no collective. After attention, each core holds ctx for
its 3 heads over all 2048 tokens; a per-head 8-core AllToAll sends each
rank the ctx slice for its own 512 tokens. The receive frame interleaves
both batch groups; the output projection contracts over the full 1536-row
frame with a host-permuted wo whose cross-group rows are zeroed, keeping
the device program SPMD-uniform. FFN is token-parallel with full streamed
weights. All matmul operands are bf16 (fp32 PSUM accumulation); softmax
skips max-subtraction and gets the denominator via a ones-augmented V
column.
"""

import numpy as np

B, S, E, H, D = 2, 2048, 768, 12, 64
F = 4 * E
NCORES = 8
TPG = 4                 # ranks per batch group
T = S // TPG            # 512 own tokens
HPC = H // TPG          # 3 heads per core
HD = HPC * D            # 192 own head dims
P = 128
EC = E // P             # 6 embed chunks
FC = F // P             # 24 ffn-hidden chunks
TC = T // P             # 4 own token chunks
KC = S // P             # 16 key chunks (full seq)
QB = 2                  # query blocks of 1024
QW = S // QB            # 1024
EPS = 1e-5
SCALE = 1.0 / float(np.sqrt(E))

_CACHE = {}


def _build_nc():
    import concourse.bass as bass
    import concourse.mybir as mybir
    import concourse.tile as tile
    from concourse import bacc
    from concourse.masks import make_identity

    dt = mybir.dt
    f32 = dt.float32
    bf16 = dt.bfloat16
    Alu = mybir.AluOpType
    Act = mybir.ActivationFunctionType
    Axis = mybir.AxisListType

    nc = bacc.Bacc(
        "TRN2",
        target_bir_lowering=False,
        debug=False,
        enable_asserts=False,
        num_devices=NCORES,
    )

    xT_in = nc.dram_tensor("xT", [E, S], bf16, kind="ExternalInput")
    xo_in = nc.dram_tensor("x_own", [T, E], f32, kind="ExternalInput")
    wq_in = nc.dram_tensor("wq", [E, HD], bf16, kind="ExternalInput")
    wk_in = nc.dram_tensor("wk", [E, HD], bf16, kind="ExternalInput")
    wv_in = nc.dram_tensor("wv", [E, HD], bf16, kind="ExternalInput")
    bq_in = nc.dram_tensor("bq", [2 * P], f32, kind="ExternalInput")
    bk_in = nc.dram_tensor("bk", [2 * P], f32, kind="ExternalInput")
    bv_in = nc.dram_tensor("bv", [HD], f32, kind="ExternalInput")
    wop_in = nc.dram_tensor("wop", [NCORES * HD, E], bf16, kind="ExternalInput")
    bo_in = nc.dram_tensor("bo", [E], f32, kind="ExternalInput")
    ln1g_in = nc.dram_tensor("ln1_g", [E], f32, kind="ExternalInput")
    ln1b_in = nc.dram_tensor("ln1_b", [E], f32, kind="ExternalInput")
    ln2g_in = nc.dram_tensor("ln2_g", [E], f32, kind="ExternalInput")
    ln2b_in = nc.dram_tensor("ln2_b", [E], f32, kind="ExternalInput")
    w1_in = nc.dram_tensor("w1", [E, F], bf16, kind="ExternalInput")
    b1_in = nc.dram_tensor("b1", [F], f32, kind="ExternalInput")
    w2_in = nc.dram_tensor("w2", [F, E], bf16, kind="ExternalInput")
    b2_in = nc.dram_tensor("b2", [E], f32, kind="ExternalInput")
    out_dram = nc.dram_tensor("out", [T, E], f32, kind="ExternalOutput")
    import os as _os
    DBG = bool(_os.environ.get("KBUILD_DEBUG"))
    if DBG:
        dbg_rs = nc.dram_tensor("dbg_rs", [P, S], bf16, kind="ExternalOutput")
        dbg_mu = nc.dram_tensor("dbg_mu", [P, S], bf16, kind="ExternalOutput")
        dbg_xh = nc.dram_tensor("dbg_xh", [P, S], bf16, kind="ExternalOutput")
        dbg_kt = nc.dram_tensor("dbg_kt", [P, S], bf16, kind="ExternalOutput")
        dbg_v3 = nc.dram_tensor("dbg_v3", [P, KC * HPC * (D + 1)], bf16, kind="ExternalOutput")
        dbg_ctx = nc.dram_tensor("dbg_ctx", [HPC, 64, S], bf16, kind="ExternalOutput")
        dbg_a2i = nc.dram_tensor("dbg_a2i", [NCORES, D, T], bf16, kind="ExternalOutput")
        dbg_a2o = nc.dram_tensor("dbg_a2o", [NCORES, D, T], bf16, kind="ExternalOutput")
        dbg_ca = nc.dram_tensor("dbg_ca", [P, 2 * EC * T], bf16, kind="ExternalOutput")
        dbg_y = nc.dram_tensor("dbg_y", [P, TC * E], f32, kind="ExternalOutput")
        dbg_y2t = nc.dram_tensor("dbg_y2t", [P, EC * T], bf16, kind="ExternalOutput")
        dbg_h = nc.dram_tensor("dbg_h", [P, FC * T], bf16, kind="ExternalOutput")

    # per-own-head AllToAll bounce buffers
    a2a_in = nc.dram_tensor("a2a_in", [HPC, NCORES, D, T], bf16)
    a2a_out = nc.dram_tensor("a2a_out", [HPC, NCORES, D, T], bf16)
    a2a_groups = [list(range(NCORES))]

    with tile.TileContext(nc) as tc:
        const_pool = tc.alloc_tile_pool(name="const", bufs=1)
        acts = tc.alloc_tile_pool(name="acts", bufs=1)
        stream = tc.alloc_tile_pool(name="stream", bufs=1)

        # ---------------- constants ----------------
        ident = const_pool.tile([P, P], bf16)
        make_identity(nc, ident)
        ones_col = const_pool.tile([P, 1], bf16)
        nc.vector.memset(ones_col, 1.0)
        ones64 = const_pool.tile([1, 64], bf16)
        nc.vector.memset(ones64, 1.0)
        eps_col = const_pool.tile([1, 1], f32)
        nc.vector.memset(eps_col, EPS)
        eps_col2 = const_pool.tile([P, 1], f32)
        nc.vector.memset(eps_col2, EPS)

        ln1g_col = const_pool.tile([P, EC], f32)
        nc.sync.dma_start(ln1g_col, ln1g_in.rearrange("(c p) -> p c", p=P))
        ln1b_col = const_pool.tile([P, EC], f32)
        nc.sync.dma_start(ln1b_col, ln1b_in.rearrange("(c p) -> p c", p=P))
        bqc = const_pool.tile([P, 2], f32)
        nc.sync.dma_start(bqc, bq_in.rearrange("(c p) -> p c", p=P))
        bkc = const_pool.tile([P, 2], f32)
        nc.sync.dma_start(bkc, bk_in.rearrange("(c p) -> p c", p=P))
        b1_col = const_pool.tile([P, FC], f32)
        nc.sync.dma_start(b1_col, b1_in.rearrange("(c p) -> p c", p=P))

        # free-axis rows replicated across partitions
        reps = {}
        for name, t_in, width in [
            ("bv", bv_in, HD), ("bo", bo_in, E), ("b2", b2_in, E),
            ("ln2_g", ln2g_in, E), ("ln2_b", ln2b_in, E),
        ]:
            row = const_pool.tile([1, width], f32, name=f"{name}_row")
            nc.sync.dma_start(row, t_in[None, :])
            rep = const_pool.tile([P, width], f32, name=f"{name}_rep")
            nc.gpsimd.partition_broadcast(rep, row)
            reps[name] = rep

        # ================ phase 1: stats, x-hat, QKV ================
        ph1_sb = tc.alloc_tile_pool(name="ph1_sb", bufs=1)
        ph1a_ps = tc.alloc_tile_pool(name="ph1a_ps", bufs=1, space="PSUM")

        xt = ph1_sb.tile([P, EC, S], bf16)
        xt_v = xT_in.rearrange("(c p) t -> p c t", p=P)
        for ec in range(EC):
            nc.sync.dma_start(xt[:, ec, :], xt_v[:, ec, :])
        xo = acts.tile([P, TC, E], f32)
        nc.sync.dma_start(xo, xo_in.rearrange("(c p) e -> p c e", p=P))

        wq_sb = ph1_sb.tile([P, EC, HD], bf16)
        nc.sync.dma_start(wq_sb, wq_in.rearrange("(c p) d -> p c d", p=P))
        wk_sb = ph1_sb.tile([P, EC, HD], bf16)
        nc.sync.dma_start(wk_sb, wk_in.rearrange("(c p) d -> p c d", p=P))
        wv_sb = ph1_sb.tile([P, EC, HD], bf16)
        nc.sync.dma_start(wv_sb, wv_in.rearrange("(c p) d -> p c d", p=P))

        # LN1 stats for all 2048 tokens: col-sums of x and x^2 via PE
        st_s = [
            ph1a_ps.tile([1, 512], f32, tag=f"sts{qb}", bufs=1, name=f"sts{qb}")
            for qb in range(4)
        ]
        st_q = [
            ph1a_ps.tile([1, 512], f32, tag=f"stq{qb}", bufs=1, name=f"stq{qb}")
            for qb in range(4)
        ]
        for ec in range(EC):
            sq = stream.tile([P, S], bf16, tag="sq", bufs=2, name="sq")
            nc.scalar.activation(sq, xt[:, ec, :], Act.Square)
            for qb in range(4):
                sl = slice(qb * 512, (qb + 1) * 512)
                nc.tensor.matmul(
                    st_s[qb], ones_col, xt[:, ec, sl],
                    start=(ec == 0), stop=(ec == EC - 1),
                )
                nc.tensor.matmul(
                    st_q[qb], ones_col, sq[:, sl],
                    start=(ec == 0), stop=(ec == EC - 1),
                )

        rs_b = ph1_sb.tile([P, S], bf16)
        murs_b = ph1_sb.tile([P, S], bf16)
        for qb in range(4):
            sl = slice(qb * 512, (qb + 1) * 512)
            mean = ph1_sb.tile([1, 512], f32, name=f"mean{qb}")
            nc.vector.tensor_scalar(mean, st_s[qb], 1.0 / E, None, Alu.mult)
            var = ph1_sb.tile([1, 512], f32, name=f"var{qb}")
            nc.vector.tensor_scalar(var, st_q[qb], 1.0 / E, None, Alu.mult)
            msq = ph1_sb.tile([1, 512], f32, name=f"msq{qb}")
            nc.vector.tensor_tensor(msq, mean, mean, Alu.mult)
            nc.vector.tensor_tensor(var, var, msq, Alu.subtract)
            lnv = ph1_sb.tile([1, 512], f32, name=f"lnv{qb}")
            nc.scalar.activation(lnv, var, Act.Ln, bias=eps_col)
            rsq = ph1_sb.tile([1, 512], f32, name=f"rsq{qb}")
            nc.scalar.activation(rsq, lnv, Act.Exp, scale=-0.5)
            rs_bf = ph1_sb.tile([1, 512], bf16, name=f"rsbf{qb}")
            nc.vector.tensor_copy(rs_bf, rsq)
            murs_bf = ph1_sb.tile([1, 512], bf16, name=f"mursbf{qb}")
            nc.vector.tensor_tensor(murs_bf, mean, rsq, Alu.mult)
            nc.gpsimd.partition_broadcast(rs_b[:, sl], rs_bf)
            nc.gpsimd.partition_broadcast(murs_b[:, sl], murs_bf)

        # x-hat^T = ((x*rs) - mu*rs) * g + b   (bf16, in-place over xt)
        xhat = xt
        for ec in range(EC):
            t1 = stream.tile([P, S], bf16, tag="xh1", bufs=2, name="xh1")
            nc.vector.tensor_tensor(t1, xt[:, ec, :], rs_b, Alu.mult)
            nc.vector.tensor_tensor(t1, t1, murs_b, Alu.subtract)
            nc.vector.tensor_scalar(
                xhat[:, ec, :], t1,
                ln1g_col[:, ec, None], ln1b_col[:, ec, None],
                Alu.mult, Alu.add,
            )

        ph1a_ps.release()
        ph1b_ps = tc.alloc_tile_pool(name="ph1b_ps", bufs=1, space="PSUM")

        # K^T and Q^T for own heads over all tokens: [HD rows, S]
        kT_a = acts.tile([P, S], bf16)
        kT_b = acts.tile([64, S], bf16)
        qT_a = acts.tile([P, S], bf16)
        qT_b = acts.tile([64, S], bf16)
        for (w_sb, bc_col, dst_a, dst_b) in (
            (wk_sb, bkc, kT_a, kT_b),
            (wq_sb, bqc, qT_a, qT_b),
        ):
            for qb in range(4):
                sl = slice(qb * 512, (qb + 1) * 512)
                psa = ph1b_ps.tile([P, 512], f32, tag="proj", bufs=2, name="proj")
                psb = ph1b_ps.tile([64, 512], f32, tag="projB", bufs=2, name="projB")
                for ec in range(EC):
                    nc.tensor.matmul(
                        psa, w_sb[:, ec, 0:P], xhat[:, ec, sl],
                        start=(ec == 0), stop=(ec == EC - 1),
                    )
                    nc.tensor.matmul(
                        psb, w_sb[:, ec, P:HD], xhat[:, ec, sl],
                        start=(ec == 0), stop=(ec == EC - 1),
                    )
                nc.vector.tensor_scalar(
                    dst_a[:, sl], psa, bc_col[:, 0, None], None, Alu.add
                )
                nc.vector.tensor_scalar(
                    dst_b[:, sl], psb, bc_col[0:64, 1, None], None, Alu.add
                )

        # V natural (per key chunk), ones-augmented: [128k, KC, HPC, D+1]
        v3 = acts.tile([P, KC, HPC, D + 1], bf16)
        for kc in range(KC):
            for i in range(HPC):
                nc.vector.memset(v3[:, kc, i, D, None], 1.0)
        for kc in range(KC):
            vp = ph1b_ps.tile([P, HD], f32, tag="vp", bufs=2, name="vp")
            tsl = slice(kc * P, (kc + 1) * P)
            for ec in range(EC):
                nc.tensor.matmul(
                    vp, xhat[:, ec, tsl], wv_sb[:, ec, :],
                    start=(ec == 0), stop=(ec == EC - 1),
                )
            for i in range(HPC):
                nc.vector.tensor_tensor(
                    v3[:, kc, i, 0:D], vp[:, i * D:(i + 1) * D],
                    reps["bv"][:, i * D:(i + 1) * D], Alu.add,
                )

        if DBG:
            nc.sync.dma_start(dbg_rs[:, :], rs_b)
            nc.sync.dma_start(dbg_mu[:, :], murs_b)
            nc.sync.dma_start(dbg_xh[:, :], xhat[:, 0, :])
            nc.sync.dma_start(dbg_kt[:, :], kT_a)
            nc.sync.dma_start(dbg_v3[:, :], v3.rearrange("p a b c -> p (a b c)"))

        ph1_sb.release()
        ph1b_ps.release()

        # ================ phase 2: attention (3 own heads) ================
        att_sb = tc.alloc_tile_pool(name="att_sb", bufs=1)
        att_ps = tc.alloc_tile_pool(name="att_ps", bufs=1, space="PSUM")

        # prefetch heavy phase-3 weights early (overlaps attention)
        wop_sb = att_sb.tile([P, 2 * EC, E], bf16)
        nc.sync.dma_start(wop_sb, wop_in.rearrange("(c p) o -> p c o", p=P))

        for i in range(HPC):
            if i == 0:
                krows, qrows = kT_a[0:64], qT_a[0:64]
            elif i == 1:
                krows, qrows = kT_a[64:128], qT_a[64:128]
            else:
                krows, qrows = kT_b[0:64], qT_b[0:64]
            ctxT = att_sb.tile([64, S], bf16, tag="ctxT", bufs=2, name="ctxT")
            for qb in range(QB):
                ctx_ps = att_ps.tile([D + 1, QW], f32, tag="ctx", bufs=2, name="ctx")
                exps = [None] * KC

                def emit_av(kc):
                    for h2 in range(2):
                        nc.tensor.matmul(
                            ctx_ps[:, h2 * 512:(h2 + 1) * 512],
                            v3[:, kc, i, :],
                            exps[kc][:, h2 * 512:(h2 + 1) * 512],
                            start=(kc == 0), stop=(kc == KC - 1),
                        )

                for kc in range(KC):
                    s_ps = att_ps.tile([P, QW], f32, tag="sps", bufs=2, name="sps")
                    for h2 in range(2):
                        nc.tensor.matmul(
                            s_ps[:, h2 * 512:(h2 + 1) * 512],
                            krows[:, kc * P:(kc + 1) * P],
                            qrows[:, qb * QW + h2 * 512: qb * QW + (h2 + 1) * 512],
                            start=True, stop=True,
                        )
                    exps[kc] = att_sb.tile([P, QW], bf16, tag="exp", bufs=3, name="exp")
                    nc.scalar.activation(exps[kc], s_ps, Act.Exp, scale=SCALE)
                    if kc >= 1:
                        emit_av(kc - 1)
                emit_av(KC - 1)
                den = att_sb.tile([1, QW], f32, tag="den", bufs=2, name="den")
                nc.vector.tensor_copy(den, ctx_ps[D:D + 1, :])
                den_f = att_sb.tile([1, QW], f32, tag="denf", bufs=2, name="denf")
                nc.vector.reciprocal_approx_fast(den_f, den)
                bc_rep = att_sb.tile([64, QW], f32, tag="bcr", bufs=2, name="bcr")
                nc.gpsimd.partition_broadcast(bc_rep, den_f)
                nc.vector.tensor_tensor(
                    ctxT[:, qb * QW:(qb + 1) * QW], ctx_ps[0:64, :], bc_rep, Alu.mult
                )
            if DBG:
                nc.sync.dma_start(dbg_ctx[i], ctxT)
            for j in range(NCORES):
                r = j % TPG
                nc.sync.dma_start(a2a_in[i, j], ctxT[:, r * T:(r + 1) * T])
            nc.gpsimd.collective_compute(
                "AllToAll", mybir.AluOpType.bypass,
                replica_groups=a2a_groups,
                ins=[a2a_in[i]],
                outs=[a2a_out[i]],
            )

        # ================ phase 3: assemble ctx, out-proj, residual =======
        # frame row j*HD + i*D + d  <->  a2a_out[i, j, d, :]
        ctx_all = att_sb.tile([P, 2 * EC, T], bf16)
        for j in range(NCORES):
            for i in range(HPC):
                row = j * HD + i * D
                cc, po = row // P, row % P
                nc.sync.dma_start(ctx_all[po:po + D, cc, :], a2a_out[i, j])

        if DBG:
            nc.sync.dma_start(dbg_ca[:, :], ctx_all.rearrange("p a b -> p (a b)"))
            for j in range(NCORES):
                bnc_i = stream.tile([64, T], bf16, tag="bnci", bufs=1, name="bnci")
                nc.sync.dma_start(bnc_i, a2a_in[0, j])
                nc.sync.dma_start(dbg_a2i[j], bnc_i)
                bnc_o = stream.tile([64, T], bf16, tag="bnco", bufs=1, name="bnco")
                nc.sync.dma_start(bnc_o, a2a_out[0, j])
                nc.sync.dma_start(dbg_a2o[j], bnc_o)
        att_ps.release()
        ph3_ps = tc.alloc_tile_pool(name="ph3_ps", bufs=1, space="PSUM")

        y_sb = acts.tile([P, TC, E], f32)
        for c in range(TC):
            tsl = slice(c * P, (c + 1) * P)
            for off, wdt in ((0, 512), (512, 256)):
                ps = ph3_ps.tile([P, 512], f32, tag="zo", bufs=2, name="zo")[:, :wdt]
                cc_order = [0, 3, 6, 9, 1, 2, 4, 5, 7, 8, 10, 11]
                for n_cc, cc in enumerate(cc_order):
                    nc.tensor.matmul(
                        ps, ctx_all[:, cc, tsl], wop_sb[:, cc, off:off + wdt],
                        start=(n_cc == 0), stop=(n_cc == 2 * EC - 1),
                    )
                osl = slice(off, off + wdt)
                nc.vector.tensor_tensor(
                    y_sb[:, c, osl], ps, reps["bo"][:, osl], Alu.add
                )
                nc.vector.tensor_tensor(
                    y_sb[:, c, osl], y_sb[:, c, osl], xo[:, c, osl], Alu.add
                )
        att_sb.release()

        if DBG:
            nc.sync.dma_start(dbg_y[:, :], y_sb.rearrange("p a b -> p (a b)"))
        # ================ phase 4: LN2 + transpose ================
        ffn_sb = tc.alloc_tile_pool(name="ffn_sb", bufs=1)
        stats2 = ffn_sb.tile([P, TC, 4], f32)
        s2 = stats2[:, :, 0]
        ss2 = stats2[:, :, 1]
        m2 = stats2[:, :, 2]
        r2 = stats2[:, :, 3]
        for c in range(TC):
            sq2 = stream.tile([P, E], f32, tag="sq2", bufs=2, name="sq2")
            nc.vector.tensor_reduce(s2[:, c, None], y_sb[:, c, :], Axis.X, Alu.add)
            nc.vector.tensor_tensor(sq2, y_sb[:, c, :], y_sb[:, c, :], Alu.mult)
            nc.vector.tensor_reduce(ss2[:, c, None], sq2, Axis.X, Alu.add)
        nc.vector.tensor_scalar(m2[:, :, None], s2[:, :, None], 1.0 / E, None, Alu.mult)
        var2 = ffn_sb.tile([P, TC], f32)
        nc.vector.tensor_scalar(var2[:, :, None], ss2[:, :, None], 1.0 / E, None, Alu.mult)
        msq2 = ffn_sb.tile([P, TC], f32)
        nc.vector.tensor_tensor(msq2[:, :, None], m2[:, :, None], m2[:, :, None], Alu.mult)
        nc.vector.tensor_tensor(var2[:, :, None], var2[:, :, None], msq2[:, :, None], Alu.subtract)
        lnv2 = ffn_sb.tile([P, TC], f32)
        nc.scalar.activation(lnv2[:, :, None], var2[:, :, None], Act.Ln, bias=eps_col2)
        nc.scalar.activation(r2[:, :, None], lnv2[:, :, None], Act.Exp, scale=-0.5)

        y2 = ffn_sb.tile([P, TC, E], bf16)
        for c in range(TC):
            nc.vector.tensor_scalar(
                y2[:, c, :], y_sb[:, c, :],
                m2[:, c, None], r2[:, c, None],
                Alu.subtract, Alu.mult,
            )
            nc.vector.tensor_tensor(y2[:, c, :], y2[:, c, :], reps["ln2_g"], Alu.mult)
            nc.vector.tensor_tensor(y2[:, c, :], y2[:, c, :], reps["ln2_b"], Alu.add)

        y2T = ffn_sb.tile([P, EC, T], bf16)
        for ec in range(EC):
            for c in range(TC):
                tps = ph3_ps.tile([P, P], bf16, tag="tp", bufs=2, name="tp")
                nc.tensor.transpose(tps, y2[:, c, ec * P:(ec + 1) * P], ident)
                nc.vector.tensor_copy(y2T[:, ec, c * P:(c + 1) * P], tps)
        ph3_ps.release()

        if DBG:
            nc.sync.dma_start(dbg_y2t[:, :], y2T.rearrange("p a b -> p (a b)"))
        # ================ phase 5: FFN ================
        ffn_ps = tc.alloc_tile_pool(name="ffn_ps", bufs=1, space="PSUM")
        hT = ffn_sb.tile([P, FC, T], bf16)
        w2_sb = ffn_sb.tile([P, FC, E], bf16)
        nc.sync.dma_start(w2_sb, w2_in.rearrange("(c p) o -> p c o", p=P))
        for fc in range(FC):
            w1b = ffn_sb.tile([P, EC, P], bf16, tag="w1b", bufs=3, name="w1b")
            nc.sync.dma_start(
                w1b, w1_in[:, fc * P:(fc + 1) * P].rearrange("(c p) h -> p c h", p=P)
            )
            hps = ffn_ps.tile([P, T], f32, tag="h", bufs=3, name="h")
            for ec in range(EC):
                nc.tensor.matmul(
                    hps, w1b[:, ec, :], y2T[:, ec, :],
                    start=(ec == 0), stop=(ec == EC - 1),
                )
            nc.scalar.activation(hT[:, fc, :], hps, Act.Gelu, bias=b1_col[:, fc, None])

        if DBG:
            nc.sync.dma_start(dbg_h[:, :], hT.rearrange("p a b -> p (a b)"))
        for c in range(TC):
            tsl = slice(c * P, (c + 1) * P)
            za = ffn_ps.tile([P, 512], f32, tag="zf1", bufs=2, name="zf1")
            zb = ffn_ps.tile([P, 256], f32, tag="zf2", bufs=2, name="zf2")
            for fc in range(FC):
                nc.tensor.matmul(
                    za, hT[:, fc, tsl], w2_sb[:, fc, 0:512],
                    start=(fc == 0), stop=(fc == FC - 1),
                )
                nc.tensor.matmul(
                    zb, hT[:, fc, tsl], w2_sb[:, fc, 512:768],
                    start=(fc == 0), stop=(fc == FC - 1),
                )
            o_sb = stream.tile([P, E], f32, tag="o", bufs=2, name="o")
            nc.vector.tensor_tensor(o_sb[:, 0:512], za, y_sb[:, c, 0:512], Alu.add)
            nc.vector.tensor_tensor(o_sb[:, 512:768], zb, y_sb[:, c, 512:768], Alu.add)
            nc.vector.tensor_tensor(o_sb, o_sb, reps["b2"], Alu.add)
            nc.sync.dma_start(out_dram[c * P:(c + 1) * P, :], o_sb)

        ffn_ps.release()
        ffn_sb.release()
        stream.release()
        acts.release()
        const_pool.release()

    nc.finalize()
    return nc


def _get_nc():
    if "nc" not in _CACHE:
        _CACHE["nc"] = _build_nc()
    return _CACHE["nc"]


def _shard_inputs(inputs):
    import ml_dtypes

    bf16 = ml_dtypes.bfloat16
    x = np.asarray(inputs["x"], dtype=np.float32)
    f = {k: np.asarray(v, dtype=np.float32) for k, v in inputs.items() if k != "x"}

    xT = [np.ascontiguousarray(x[g].T).astype(bf16) for g in range(B)]
    wo = f["wo"]

    in_maps = []
    for c in range(NCORES):
        g, r = c // TPG, c % TPG
        hsl = slice(HD * r, HD * r + HD)

        wop = np.zeros((NCORES * HD, E), np.float32)
        for j in range(NCORES):
            if j // TPG == g:
                wop[j * HD:(j + 1) * HD] = wo[HD * (j % TPG): HD * (j % TPG) + HD]

        def pad(b):
            v = np.zeros(2 * P, np.float32)
            v[:HD] = b
            return v

        m = {
            "xT": xT[g],
            "x_own": np.ascontiguousarray(x[g, r * T:(r + 1) * T]),
            "wq": np.ascontiguousarray(f["wq"][:, hsl]).astype(bf16),
            "wk": np.ascontiguousarray(f["wk"][:, hsl]).astype(bf16),
            "wv": np.ascontiguousarray(f["wv"][:, hsl]).astype(bf16),
            "bq": pad(f["bq"][hsl]),
            "bk": pad(f["bk"][hsl]),
            "bv": np.ascontiguousarray(f["bv"][hsl]),
            "wop": wop.astype(bf16),
            "bo": f["bo"],
            "ln1_g": f["ln1_g"], "ln1_b": f["ln1_b"],
            "ln2_g": f["ln2_g"], "ln2_b": f["ln2_b"],
            "w1": f["w1"].astype(bf16), "b1": f["b1"],
            "w2": f["w2"].astype(bf16), "b2": f["b2"],
        }
        in_maps.append(m)
    return in_maps


def kernel(**inputs):
    from concourse.bass_utils import run_bass_kernel_spmd

    nc = _get_nc()
    in_maps = _shard_inputs(inputs)
    res = run_bass_kernel_spmd(nc, in_maps, core_ids=list(range(NCORES)))
    _CACHE["last_results"] = res
    out = np.empty((B, S, E), np.float32)
    for c in range(NCORES):
        g, r = c // TPG, c % TPG
        out[g, r * T:(r + 1) * T, :] = res.results[c]["out"]
    return out


# revision 24
# speedup vs baseline: 1.1353x; 1.0239x over previous
"""Trainium2 Bass kernel for a dense transformer block (B=2, S=2048, E=768, H=12).

Sharding: 8 cores = 2 batch groups x 4 ranks. Head-parallel attention:
core (g, r) owns heads [3r, 3r+3) of batch element g and token rows
[512r, 512r+512) for everything token-local (residuals, LN2, FFN, output).

The host replicates x^T (bf16) across each batch group, so LN1 stats and
Q/K/V projections for the core's own heads over the FULL sequence start
immediately with no collective. After attention, each core holds ctx for
its 3 heads over all 2048 tokens; a per-head 8-core AllToAll sends each
rank the ctx slice for its own 512 tokens. The receive frame interleaves
both batch groups; the output projection contracts over the full 1536-row
frame with a host-permuted wo whose cross-group rows are zeroed, keeping
the device program SPMD-uniform. FFN is token-parallel with full streamed
weights. All matmul operands are bf16 (fp32 PSUM accumulation); softmax
skips max-subtraction and gets the denominator via a ones-augmented V
column.
"""

import numpy as np

B, S, E, H, D = 2, 2048, 768, 12, 64
F = 4 * E
NCORES = 8
TPG = 4                 # ranks per batch group
T = S // TPG            # 512 own tokens
HPC = H // TPG          # 3 heads per core
HD = HPC * D            # 192 own head dims
P = 128
EC = E // P             # 6 embed chunks
FC = F // P             # 24 ffn-hidden chunks
TC = T // P             # 4 own token chunks
KC = S // P             # 16 key chunks (full seq)
QB = 2                  # query blocks of 1024
QW = S // QB            # 1024
EPS = 1e-5
SCALE = 1.0 / float(np.sqrt(E))

_CACHE = {}


def _build_nc():
    import concourse.bass as bass
    import concourse.mybir as mybir
    import concourse.tile as tile
    from concourse import bacc
    from concourse.masks import make_identity

    dt = mybir.dt
    f32 = dt.float32
    bf16 = dt.bfloat16
    Alu = mybir.AluOpType
    Act = mybir.ActivationFunctionType
    Axis = mybir.AxisListType

    nc = bacc.Bacc(
        "TRN2",
        target_bir_lowering=False,
        debug=False,
        enable_asserts=False,
        num_devices=NCORES,
    )

    xT_in = nc.dram_tensor("xT", [E, S], bf16, kind="ExternalInput")
    xo_in = nc.dram_tensor("x_own", [T, E], f32, kind="ExternalInput")
    wq_in = nc.dram_tensor("wq", [E, HD], bf16, kind="ExternalInput")
    wk_in = nc.dram_tensor("wk", [E, HD], bf16, kind="ExternalInput")
    wv_in = nc.dram_tensor("wv", [E, HD], bf16, kind="ExternalInput")
    bq_in = nc.dram_tensor("bq", [2 * P], f32, kind="ExternalInput")
    bk_in = nc.dram_tensor("bk", [2 * P], f32, kind="ExternalInput")
    bv_in = nc.dram_tensor("bv", [HD], f32, kind="ExternalInput")
    wop_in = nc.dram_tensor("wop", [NCORES * HD, E], bf16, kind="ExternalInput")
    bo_in = nc.dram_tensor("bo", [E], f32, kind="ExternalInput")
    ln1g_in = nc.dram_tensor("ln1_g", [E], f32, kind="ExternalInput")
    ln1b_in = nc.dram_tensor("ln1_b", [E], f32, kind="ExternalInput")
    ln2g_in = nc.dram_tensor("ln2_g", [E], f32, kind="ExternalInput")
    ln2b_in = nc.dram_tensor("ln2_b", [E], f32, kind="ExternalInput")
    w1_in = nc.dram_tensor("w1", [E, F], bf16, kind="ExternalInput")
    b1_in = nc.dram_tensor("b1", [F], f32, kind="ExternalInput")
    w2_in = nc.dram_tensor("w2", [F, E], bf16, kind="ExternalInput")
    b2_in = nc.dram_tensor("b2", [E], f32, kind="ExternalInput")
    out_dram = nc.dram_tensor("out", [T, E], f32, kind="ExternalOutput")
    import os as _os
    DBG = bool(_os.environ.get("KBUILD_DEBUG"))
    if DBG:
        dbg_rs = nc.dram_tensor("dbg_rs", [P, S], bf16, kind="ExternalOutput")
        dbg_mu = nc.dram_tensor("dbg_mu", [P, S], bf16, kind="ExternalOutput")
        dbg_xh = nc.dram_tensor("dbg_xh", [P, S], bf16, kind="ExternalOutput")
        dbg_kt = nc.dram_tensor("dbg_kt", [P, S], bf16, kind="ExternalOutput")
        dbg_v3 = nc.dram_tensor("dbg_v3", [P, KC * HPC * (D + 1)], bf16, kind="ExternalOutput")
        dbg_ctx = nc.dram_tensor("dbg_ctx", [HPC, 64, S], bf16, kind="ExternalOutput")
        dbg_a2i = nc.dram_tensor("dbg_a2i", [NCORES, D, T], bf16, kind="ExternalOutput")
        dbg_a2o = nc.dram_tensor("dbg_a2o", [NCORES, D, T], bf16, kind="ExternalOutput")
        dbg_ca = nc.dram_tensor("dbg_ca", [P, 2 * EC * T], bf16, kind="ExternalOutput")
        dbg_y = nc.dram_tensor("dbg_y", [P, TC * E], f32, kind="ExternalOutput")
        dbg_y2t = nc.dram_tensor("dbg_y2t", [P, EC * T], bf16, kind="ExternalOutput")
        dbg_h = nc.dram_tensor("dbg_h", [P, FC * T], bf16, kind="ExternalOutput")

    # per-own-head AllToAll bounce buffers
    a2a_in = nc.dram_tensor("a2a_in", [HPC, NCORES, D, T], bf16)
    a2a_out = nc.dram_tensor("a2a_out", [HPC, NCORES, D, T], bf16)
    a2a_groups = [list(range(NCORES))]

    with tile.TileContext(nc) as tc:
        const_pool = tc.alloc_tile_pool(name="const", bufs=1)
        acts = tc.alloc_tile_pool(name="acts", bufs=1)
        stream = tc.alloc_tile_pool(name="stream", bufs=1)

        # ---------------- constants ----------------
        ident = const_pool.tile([P, P], bf16)
        make_identity(nc, ident)
        ones_col = const_pool.tile([P, 1], bf16)
        nc.vector.memset(ones_col, 1.0)
        ones64 = const_pool.tile([1, 64], bf16)
        nc.vector.memset(ones64, 1.0)
        eps_col = const_pool.tile([1, 1], f32)
        nc.vector.memset(eps_col, EPS)
        eps_col2 = const_pool.tile([P, 1], f32)
        nc.vector.memset(eps_col2, EPS)

        ln1g_col = const_pool.tile([P, EC], f32)
        nc.sync.dma_start(ln1g_col, ln1g_in.rearrange("(c p) -> p c", p=P))
        ln1b_col = const_pool.tile([P, EC], f32)
        nc.sync.dma_start(ln1b_col, ln1b_in.rearrange("(c p) -> p c", p=P))
        bqc = const_pool.tile([P, 2], f32)
        nc.sync.dma_start(bqc, bq_in.rearrange("(c p) -> p c", p=P))
        bkc = const_pool.tile([P, 2], f32)
        nc.sync.dma_start(bkc, bk_in.rearrange("(c p) -> p c", p=P))
        b1_col = const_pool.tile([P, FC], f32)
        nc.sync.dma_start(b1_col, b1_in.rearrange("(c p) -> p c", p=P))

        # free-axis rows replicated across partitions
        reps = {}
        for name, t_in, width in [
            ("bv", bv_in, HD), ("bo", bo_in, E), ("b2", b2_in, E),
            ("ln2_g", ln2g_in, E), ("ln2_b", ln2b_in, E),
        ]:
            row = const_pool.tile([1, width], f32, name=f"{name}_row")
            nc.sync.dma_start(row, t_in[None, :])
            rep = const_pool.tile([P, width], f32, name=f"{name}_rep")
            nc.gpsimd.partition_broadcast(rep, row)
            reps[name] = rep

        # ================ phase 1: stats, x-hat, QKV ================
        ph1_sb = tc.alloc_tile_pool(name="ph1_sb", bufs=1)
        ph1a_ps = tc.alloc_tile_pool(name="ph1a_ps", bufs=1, space="PSUM")

        xt = ph1_sb.tile([P, EC, S], bf16)
        xt_v = xT_in.rearrange("(c p) t -> p c t", p=P)
        for ec in range(EC):
            nc.sync.dma_start(xt[:, ec, :], xt_v[:, ec, :])
        xo = acts.tile([P, TC, E], f32)
        nc.sync.dma_start(xo, xo_in.rearrange("(c p) e -> p c e", p=P))

        wq_sb = ph1_sb.tile([P, EC, HD], bf16)
        nc.sync.dma_start(wq_sb, wq_in.rearrange("(c p) d -> p c d", p=P))
        wk_sb = ph1_sb.tile([P, EC, HD], bf16)
        nc.sync.dma_start(wk_sb, wk_in.rearrange("(c p) d -> p c d", p=P))
        wv_sb = ph1_sb.tile([P, EC, HD], bf16)
        nc.sync.dma_start(wv_sb, wv_in.rearrange("(c p) d -> p c d", p=P))

        # LN1 stats for all 2048 tokens: col-sums of x and x^2 via PE
        st_s = [
            ph1a_ps.tile([1, 512], f32, tag=f"sts{qb}", bufs=1, name=f"sts{qb}")
            for qb in range(4)
        ]
        st_q = [
            ph1a_ps.tile([1, 512], f32, tag=f"stq{qb}", bufs=1, name=f"stq{qb}")
            for qb in range(4)
        ]
        for ec in range(EC):
            for qb in range(4):
                sl = slice(qb * 512, (qb + 1) * 512)
                nc.tensor.matmul(
                    st_s[qb], ones_col, xt[:, ec, sl],
                    start=(ec == 0), stop=(ec == EC - 1),
                )
        for ec in range(EC):
            sq = stream.tile([P, S], bf16, tag="sq", bufs=2, name="sq")
            nc.scalar.activation(sq, xt[:, ec, :], Act.Square)
            for qb in range(4):
                sl = slice(qb * 512, (qb + 1) * 512)
                nc.tensor.matmul(
                    st_q[qb], ones_col, sq[:, sl],
                    start=(ec == 0), stop=(ec == EC - 1),
                )

        rs_b = ph1_sb.tile([P, S], bf16)
        murs_b = ph1_sb.tile([P, S], bf16)
        for qb in range(4):
            sl = slice(qb * 512, (qb + 1) * 512)
            mean = ph1_sb.tile([1, 512], f32, name=f"mean{qb}")
            nc.vector.tensor_scalar(mean, st_s[qb], 1.0 / E, None, Alu.mult)
            var = ph1_sb.tile([1, 512], f32, name=f"var{qb}")
            nc.vector.tensor_scalar(var, st_q[qb], 1.0 / E, None, Alu.mult)
            msq = ph1_sb.tile([1, 512], f32, name=f"msq{qb}")
            nc.vector.tensor_tensor(msq, mean, mean, Alu.mult)
            nc.vector.tensor_tensor(var, var, msq, Alu.subtract)
            lnv = ph1_sb.tile([1, 512], f32, name=f"lnv{qb}")
            nc.scalar.activation(lnv, var, Act.Ln, bias=eps_col)
            rsq = ph1_sb.tile([1, 512], f32, name=f"rsq{qb}")
            nc.scalar.activation(rsq, lnv, Act.Exp, scale=-0.5)
            rs_bf = ph1_sb.tile([1, 512], bf16, name=f"rsbf{qb}")
            nc.vector.tensor_copy(rs_bf, rsq)
            murs_bf = ph1_sb.tile([1, 512], bf16, name=f"mursbf{qb}")
            nc.vector.tensor_tensor(murs_bf, mean, rsq, Alu.mult)
            nc.gpsimd.partition_broadcast(rs_b[:, sl], rs_bf)
            nc.gpsimd.partition_broadcast(murs_b[:, sl], murs_bf)

        # x-hat^T = ((x*rs) - mu*rs) * g + b   (bf16, in-place over xt)
        xhat = xt
        for ec in range(EC):
            t1 = stream.tile([P, S], bf16, tag="xh1", bufs=2, name="xh1")
            nc.vector.tensor_tensor(t1, xt[:, ec, :], rs_b, Alu.mult)
            nc.vector.tensor_tensor(t1, t1, murs_b, Alu.subtract)
            nc.vector.tensor_scalar(
                xhat[:, ec, :], t1,
                ln1g_col[:, ec, None], ln1b_col[:, ec, None],
                Alu.mult, Alu.add,
            )

        ph1a_ps.release()
        ph1b_ps = tc.alloc_tile_pool(name="ph1b_ps", bufs=1, space="PSUM")

        # K^T and Q^T for own heads over all tokens: [HD rows, S]
        kT_a = acts.tile([P, S], bf16)
        kT_b = acts.tile([64, S], bf16)
        qT_a = acts.tile([P, S], bf16)
        qT_b = acts.tile([64, S], bf16)
        for (w_sb, bc_col, dst_a, dst_b) in (
            (wk_sb, bkc, kT_a, kT_b),
            (wq_sb, bqc, qT_a, qT_b),
        ):
            for qb in range(4):
                sl = slice(qb * 512, (qb + 1) * 512)
                psa = ph1b_ps.tile([P, 512], f32, tag="proj", bufs=2, name="proj")
                psb = ph1b_ps.tile([64, 512], f32, tag="projB", bufs=2, name="projB")
                for ec in range(EC):
                    nc.tensor.matmul(
                        psa, w_sb[:, ec, 0:P], xhat[:, ec, sl],
                        start=(ec == 0), stop=(ec == EC - 1),
                    )
                    nc.tensor.matmul(
                        psb, w_sb[:, ec, P:HD], xhat[:, ec, sl],
                        start=(ec == 0), stop=(ec == EC - 1),
                    )
                nc.vector.tensor_scalar(
                    dst_a[:, sl], psa, bc_col[:, 0, None], None, Alu.add
                )
                nc.vector.tensor_scalar(
                    dst_b[:, sl], psb, bc_col[0:64, 1, None], None, Alu.add
                )

        # V natural (per key chunk), ones-augmented: [128k, KC, HPC, D+1]
        v3 = acts.tile([P, KC, HPC, D + 1], bf16)
        for kc in range(KC):
            for i in range(HPC):
                nc.vector.memset(v3[:, kc, i, D, None], 1.0)
        for kc in range(KC):
            vp = ph1b_ps.tile([P, HD], f32, tag="vp", bufs=2, name="vp")
            tsl = slice(kc * P, (kc + 1) * P)
            for ec in range(EC):
                nc.tensor.matmul(
                    vp, xhat[:, ec, tsl], wv_sb[:, ec, :],
                    start=(ec == 0), stop=(ec == EC - 1),
                )
            for i in range(HPC):
                nc.vector.tensor_tensor(
                    v3[:, kc, i, 0:D], vp[:, i * D:(i + 1) * D],
                    reps["bv"][:, i * D:(i + 1) * D], Alu.add,
                )

        if DBG:
            nc.sync.dma_start(dbg_rs[:, :], rs_b)
            nc.sync.dma_start(dbg_mu[:, :], murs_b)
            nc.sync.dma_start(dbg_xh[:, :], xhat[:, 0, :])
            nc.sync.dma_start(dbg_kt[:, :], kT_a)
            nc.sync.dma_start(dbg_v3[:, :], v3.rearrange("p a b c -> p (a b c)"))

        ph1_sb.release()
        ph1b_ps.release()

        # ================ phase 2: attention (3 own heads) ================
        att_sb = tc.alloc_tile_pool(name="att_sb", bufs=1)
        att_ps = tc.alloc_tile_pool(name="att_ps", bufs=1, space="PSUM")

        # prefetch heavy phase-3 weights early (overlaps attention)
        wop_sb = att_sb.tile([P, 2 * EC, E], bf16)
        nc.sync.dma_start(wop_sb, wop_in.rearrange("(c p) o -> p c o", p=P))

        for i in range(HPC):
            if i == 0:
                krows, qrows = kT_a[0:64], qT_a[0:64]
            elif i == 1:
                krows, qrows = kT_a[64:128], qT_a[64:128]
            else:
                krows, qrows = kT_b[0:64], qT_b[0:64]
            ctxT = att_sb.tile([64, S], bf16, tag="ctxT", bufs=2, name="ctxT")
            for qb in range(QB):
                ctx_ps = att_ps.tile([D + 1, QW], f32, tag="ctx", bufs=2, name="ctx")
                exps = [None] * KC

                def emit_av(kc):
                    for h2 in range(2):
                        nc.tensor.matmul(
                            ctx_ps[:, h2 * 512:(h2 + 1) * 512],
                            v3[:, kc, i, :],
                            exps[kc][:, h2 * 512:(h2 + 1) * 512],
                            start=(kc == 0), stop=(kc == KC - 1),
                        )

                for kc in range(KC):
                    s_ps = att_ps.tile([P, QW], f32, tag="sps", bufs=2, name="sps")
                    for h2 in range(2):
                        nc.tensor.matmul(
                            s_ps[:, h2 * 512:(h2 + 1) * 512],
                            krows[:, kc * P:(kc + 1) * P],
                            qrows[:, qb * QW + h2 * 512: qb * QW + (h2 + 1) * 512],
                            start=True, stop=True,
                        )
                    exps[kc] = att_sb.tile([P, QW], bf16, tag="exp", bufs=3, name="exp")
                    nc.scalar.activation(exps[kc], s_ps, Act.Exp, scale=SCALE)
                    if kc >= 1:
                        emit_av(kc - 1)
                emit_av(KC - 1)
                den = att_sb.tile([1, QW], f32, tag="den", bufs=2, name="den")
                nc.vector.tensor_copy(den, ctx_ps[D:D + 1, :])
                den_f = att_sb.tile([1, QW], f32, tag="denf", bufs=2, name="denf")
                nc.vector.reciprocal_approx_fast(den_f, den)
                bc_rep = att_sb.tile([64, QW], f32, tag="bcr", bufs=2, name="bcr")
                nc.gpsimd.partition_broadcast(bc_rep, den_f)
                nc.vector.tensor_tensor(
                    ctxT[:, qb * QW:(qb + 1) * QW], ctx_ps[0:64, :], bc_rep, Alu.mult
                )
            if DBG:
                nc.sync.dma_start(dbg_ctx[i], ctxT)
            for j in range(NCORES):
                r = j % TPG
                nc.sync.dma_start(a2a_in[i, j], ctxT[:, r * T:(r + 1) * T])
            nc.gpsimd.collective_compute(
                "AllToAll", mybir.AluOpType.bypass,
                replica_groups=a2a_groups,
                ins=[a2a_in[i]],
                outs=[a2a_out[i]],
            )

        # ================ phase 3: assemble ctx, out-proj, residual =======
        # frame row j*HD + i*D + d  <->  a2a_out[i, j, d, :]
        ctx_all = att_sb.tile([P, 2 * EC, T], bf16)
        for j in range(NCORES):
            for i in range(HPC):
                row = j * HD + i * D
                cc, po = row // P, row % P
                nc.sync.dma_start(ctx_all[po:po + D, cc, :], a2a_out[i, j])

        if DBG:
            nc.sync.dma_start(dbg_ca[:, :], ctx_all.rearrange("p a b -> p (a b)"))
            for j in range(NCORES):
                bnc_i = stream.tile([64, T], bf16, tag="bnci", bufs=1, name="bnci")
                nc.sync.dma_start(bnc_i, a2a_in[0, j])
                nc.sync.dma_start(dbg_a2i[j], bnc_i)
                bnc_o = stream.tile([64, T], bf16, tag="bnco", bufs=1, name="bnco")
                nc.sync.dma_start(bnc_o, a2a_out[0, j])
                nc.sync.dma_start(dbg_a2o[j], bnc_o)
        att_ps.release()
        ph3_ps = tc.alloc_tile_pool(name="ph3_ps", bufs=1, space="PSUM")

        y_sb = acts.tile([P, TC, E], f32)
        for c in range(TC):
            tsl = slice(c * P, (c + 1) * P)
            for off, wdt in ((0, 512), (512, 256)):
                ps = ph3_ps.tile([P, 512], f32, tag="zo", bufs=2, name="zo")[:, :wdt]
                cc_order = [0, 3, 6, 9, 1, 2, 4, 5, 7, 8, 10, 11]
                for n_cc, cc in enumerate(cc_order):
                    nc.tensor.matmul(
                        ps, ctx_all[:, cc, tsl], wop_sb[:, cc, off:off + wdt],
                        start=(n_cc == 0), stop=(n_cc == 2 * EC - 1),
                    )
                osl = slice(off, off + wdt)
                nc.vector.tensor_tensor(
                    y_sb[:, c, osl], ps, reps["bo"][:, osl], Alu.add
                )
                nc.vector.tensor_tensor(
                    y_sb[:, c, osl], y_sb[:, c, osl], xo[:, c, osl], Alu.add
                )
        att_sb.release()

        if DBG:
            nc.sync.dma_start(dbg_y[:, :], y_sb.rearrange("p a b -> p (a b)"))
        # ================ phase 4: LN2 + transpose ================
        ffn_sb = tc.alloc_tile_pool(name="ffn_sb", bufs=1)
        stats2 = ffn_sb.tile([P, TC, 4], f32)
        s2 = stats2[:, :, 0]
        ss2 = stats2[:, :, 1]
        m2 = stats2[:, :, 2]
        r2 = stats2[:, :, 3]
        y2 = ffn_sb.tile([P, TC, E], bf16)
        y2T = ffn_sb.tile([P, EC, T], bf16)
        var2 = ffn_sb.tile([P, TC], f32)
        msq2 = ffn_sb.tile([P, TC], f32)
        lnv2 = ffn_sb.tile([P, TC], f32)
        for c in range(TC):
            sq2 = stream.tile([P, E], f32, tag="sq2", bufs=2, name="sq2")
            nc.vector.tensor_reduce(s2[:, c, None], y_sb[:, c, :], Axis.X, Alu.add)
            nc.scalar.activation(sq2, y_sb[:, c, :], Act.Square)
            nc.vector.tensor_reduce(ss2[:, c, None], sq2, Axis.X, Alu.add)
            nc.vector.tensor_scalar(m2[:, c, None], s2[:, c, None], 1.0 / E, None, Alu.mult)
            nc.vector.tensor_scalar(var2[:, c, None], ss2[:, c, None], 1.0 / E, None, Alu.mult)
            nc.vector.tensor_tensor(msq2[:, c, None], m2[:, c, None], m2[:, c, None], Alu.mult)
            nc.vector.tensor_tensor(var2[:, c, None], var2[:, c, None], msq2[:, c, None], Alu.subtract)
            nc.scalar.activation(lnv2[:, c, None], var2[:, c, None], Act.Ln, bias=eps_col2)
            nc.scalar.activation(r2[:, c, None], lnv2[:, c, None], Act.Exp, scale=-0.5)
            nc.vector.tensor_scalar(
                y2[:, c, :], y_sb[:, c, :],
                m2[:, c, None], r2[:, c, None],
                Alu.subtract, Alu.mult,
            )
            nc.vector.tensor_tensor(y2[:, c, :], y2[:, c, :], reps["ln2_g"], Alu.mult)
            nc.vector.tensor_tensor(y2[:, c, :], y2[:, c, :], reps["ln2_b"], Alu.add)
            for ec in range(EC):
                tps = ph3_ps.tile([P, P], bf16, tag="tp", bufs=2, name="tp")
                nc.tensor.transpose(tps, y2[:, c, ec * P:(ec + 1) * P], ident)
                nc.vector.tensor_copy(y2T[:, ec, c * P:(c + 1) * P], tps)
        ph3_ps.release()

        if DBG:
            nc.sync.dma_start(dbg_y2t[:, :], y2T.rearrange("p a b -> p (a b)"))
        # ================ phase 5: FFN ================
        ffn_ps = tc.alloc_tile_pool(name="ffn_ps", bufs=1, space="PSUM")
        hT = ffn_sb.tile([P, FC, T], bf16)
        w2_sb = ffn_sb.tile([P, FC, E], bf16)
        nc.sync.dma_start(w2_sb, w2_in.rearrange("(c p) o -> p c o", p=P))
        for fc in range(FC):
            w1b = ffn_sb.tile([P, EC, P], bf16, tag="w1b", bufs=3, name="w1b")
            nc.sync.dma_start(
                w1b, w1_in[:, fc * P:(fc + 1) * P].rearrange("(c p) h -> p c h", p=P)
            )
            hps = ffn_ps.tile([P, T], f32, tag="h", bufs=3, name="h")
            for ec in range(EC):
                nc.tensor.matmul(
                    hps, w1b[:, ec, :], y2T[:, ec, :],
                    start=(ec == 0), stop=(ec == EC - 1),
                )
            nc.scalar.activation(hT[:, fc, :], hps, Act.Gelu, bias=b1_col[:, fc, None])

        if DBG:
            nc.sync.dma_start(dbg_h[:, :], hT.rearrange("p a b -> p (a b)"))
        for c in range(TC):
            tsl = slice(c * P, (c + 1) * P)
            za = ffn_ps.tile([P, 512], f32, tag="zf1", bufs=2, name="zf1")
            zb = ffn_ps.tile([P, 256], f32, tag="zf2", bufs=2, name="zf2")
            for fc in range(FC):
                nc.tensor.matmul(
                    za, hT[:, fc, tsl], w2_sb[:, fc, 0:512],
                    start=(fc == 0), stop=(fc == FC - 1),
                )
                nc.tensor.matmul(
                    zb, hT[:, fc, tsl], w2_sb[:, fc, 512:768],
                    start=(fc == 0), stop=(fc == FC - 1),
                )
            o_sb = stream.tile([P, E], f32, tag="o", bufs=2, name="o")
            nc.vector.tensor_tensor(o_sb[:, 0:512], za, y_sb[:, c, 0:512], Alu.add)
            nc.vector.tensor_tensor(o_sb[:, 512:768], zb, y_sb[:, c, 512:768], Alu.add)
            nc.vector.tensor_tensor(o_sb, o_sb, reps["b2"], Alu.add)
            nc.sync.dma_start(out_dram[c * P:(c + 1) * P, :], o_sb)

        ffn_ps.release()
        ffn_sb.release()
        stream.release()
        acts.release()
        const_pool.release()

    nc.finalize()
    return nc


def _get_nc():
    if "nc" not in _CACHE:
        _CACHE["nc"] = _build_nc()
    return _CACHE["nc"]


def _shard_inputs(inputs):
    import ml_dtypes

    bf16 = ml_dtypes.bfloat16
    x = np.asarray(inputs["x"], dtype=np.float32)
    f = {k: np.asarray(v, dtype=np.float32) for k, v in inputs.items() if k != "x"}

    xT = [np.ascontiguousarray(x[g].T).astype(bf16) for g in range(B)]
    wo = f["wo"]

    in_maps = []
    for c in range(NCORES):
        g, r = c // TPG, c % TPG
        hsl = slice(HD * r, HD * r + HD)

        wop = np.zeros((NCORES * HD, E), np.float32)
        for j in range(NCORES):
            if j // TPG == g:
                wop[j * HD:(j + 1) * HD] = wo[HD * (j % TPG): HD * (j % TPG) + HD]

        def pad(b):
            v = np.zeros(2 * P, np.float32)
            v[:HD] = b
            return v

        m = {
            "xT": xT[g],
            "x_own": np.ascontiguousarray(x[g, r * T:(r + 1) * T]),
            "wq": np.ascontiguousarray(f["wq"][:, hsl]).astype(bf16),
            "wk": np.ascontiguousarray(f["wk"][:, hsl]).astype(bf16),
            "wv": np.ascontiguousarray(f["wv"][:, hsl]).astype(bf16),
            "bq": pad(f["bq"][hsl]),
            "bk": pad(f["bk"][hsl]),
            "bv": np.ascontiguousarray(f["bv"][hsl]),
            "wop": wop.astype(bf16),
            "bo": f["bo"],
            "ln1_g": f["ln1_g"], "ln1_b": f["ln1_b"],
            "ln2_g": f["ln2_g"], "ln2_b": f["ln2_b"],
            "w1": f["w1"].astype(bf16), "b1": f["b1"],
            "w2": f["w2"].astype(bf16), "b2": f["b2"],
        }
        in_maps.append(m)
    return in_maps


def kernel(**inputs):
    from concourse.bass_utils import run_bass_kernel_spmd

    nc = _get_nc()
    in_maps = _shard_inputs(inputs)
    res = run_bass_kernel_spmd(nc, in_maps, core_ids=list(range(NCORES)))
    _CACHE["last_results"] = res
    out = np.empty((B, S, E), np.float32)
    for c in range(NCORES):
        g, r = c // TPG, c % TPG
        out[g, r * T:(r + 1) * T, :] = res.results[c]["out"]
    return out


# revision 25
# speedup vs baseline: 1.1900x; 1.0482x over previous
"""Trainium2 Bass kernel for a dense transformer block (B=2, S=2048, E=768, H=12).

Sharding: 8 cores = 2 batch groups x 4 ranks. Head-parallel attention:
core (g, r) owns heads [3r, 3r+3) of batch element g and token rows
[512r, 512r+512) for everything token-local (residuals, LN2, FFN, output).

The host replicates x^T (bf16) across each batch group, so LN1 stats and
Q/K/V projections for the core's own heads over the FULL sequence start
immediately with no collective. After attention, each core holds ctx for
its 3 heads over all 2048 tokens; a per-head 8-core AllToAll sends each
rank the ctx slice for its own 512 tokens. The receive frame interleaves
both batch groups; the output projection contracts over the full 1536-row
frame with a host-permuted wo whose cross-group rows are zeroed, keeping
the device program SPMD-uniform. FFN is token-parallel with full streamed
weights. All matmul operands are bf16 (fp32 PSUM accumulation); softmax
skips max-subtraction and gets the denominator via a ones-augmented V
column.
"""

import numpy as np

B, S, E, H, D = 2, 2048, 768, 12, 64
F = 4 * E
NCORES = 8
TPG = 4                 # ranks per batch group
T = S // TPG            # 512 own tokens
HPC = H // TPG          # 3 heads per core
HD = HPC * D            # 192 own head dims
P = 128
EC = E // P             # 6 embed chunks
FC = F // P             # 24 ffn-hidden chunks
TC = T // P             # 4 own token chunks
KC = S // P             # 16 key chunks (full seq)
QB = 2                  # query blocks of 1024
QW = S // QB            # 1024
EPS = 1e-5
SCALE = 1.0 / float(np.sqrt(E))

_CACHE = {}


def _build_nc():
    import concourse.bass as bass
    import concourse.mybir as mybir
    import concourse.tile as tile
    from concourse import bacc
    from concourse.masks import make_identity

    dt = mybir.dt
    f32 = dt.float32
    bf16 = dt.bfloat16
    Alu = mybir.AluOpType
    Act = mybir.ActivationFunctionType
    Axis = mybir.AxisListType

    nc = bacc.Bacc(
        "TRN2",
        target_bir_lowering=False,
        debug=False,
        enable_asserts=False,
        num_devices=NCORES,
    )

    xT_in = nc.dram_tensor("xT", [E, S], bf16, kind="ExternalInput")
    xo_in = nc.dram_tensor("x_own", [T, E], f32, kind="ExternalInput")
    wq_in = nc.dram_tensor("wq", [E, HD], bf16, kind="ExternalInput")
    wk_in = nc.dram_tensor("wk", [E, HD], bf16, kind="ExternalInput")
    wv_in = nc.dram_tensor("wv", [E, HD], bf16, kind="ExternalInput")
    bq_in = nc.dram_tensor("bq", [2 * P], f32, kind="ExternalInput")
    bk_in = nc.dram_tensor("bk", [2 * P], f32, kind="ExternalInput")
    bv_in = nc.dram_tensor("bv", [HD], f32, kind="ExternalInput")
    wop_in = nc.dram_tensor("wop", [NCORES * HD, E], bf16, kind="ExternalInput")
    bo_in = nc.dram_tensor("bo", [E], f32, kind="ExternalInput")
    ln1g_in = nc.dram_tensor("ln1_g", [E], f32, kind="ExternalInput")
    ln1b_in = nc.dram_tensor("ln1_b", [E], f32, kind="ExternalInput")
    ln2g_in = nc.dram_tensor("ln2_g", [E], f32, kind="ExternalInput")
    ln2b_in = nc.dram_tensor("ln2_b", [E], f32, kind="ExternalInput")
    w1_in = nc.dram_tensor("w1", [E, F], bf16, kind="ExternalInput")
    b1_in = nc.dram_tensor("b1", [F], f32, kind="ExternalInput")
    w2_in = nc.dram_tensor("w2", [F, E], bf16, kind="ExternalInput")
    b2_in = nc.dram_tensor("b2", [E], f32, kind="ExternalInput")
    out_dram = nc.dram_tensor("out", [T, E], f32, kind="ExternalOutput")
    import os as _os
    DBG = bool(_os.environ.get("KBUILD_DEBUG"))
    if DBG:
        dbg_rs = nc.dram_tensor("dbg_rs", [P, S], bf16, kind="ExternalOutput")
        dbg_mu = nc.dram_tensor("dbg_mu", [P, S], bf16, kind="ExternalOutput")
        dbg_xh = nc.dram_tensor("dbg_xh", [P, S], bf16, kind="ExternalOutput")
        dbg_kt = nc.dram_tensor("dbg_kt", [P, S], bf16, kind="ExternalOutput")
        dbg_v3 = nc.dram_tensor("dbg_v3", [P, KC * HPC * (D + 1)], bf16, kind="ExternalOutput")
        dbg_ctx = nc.dram_tensor("dbg_ctx", [HPC, 64, S], bf16, kind="ExternalOutput")
        dbg_a2i = nc.dram_tensor("dbg_a2i", [NCORES, D, T], bf16, kind="ExternalOutput")
        dbg_a2o = nc.dram_tensor("dbg_a2o", [NCORES, D, T], bf16, kind="ExternalOutput")
        dbg_ca = nc.dram_tensor("dbg_ca", [P, 2 * EC * T], bf16, kind="ExternalOutput")
        dbg_y = nc.dram_tensor("dbg_y", [P, TC * E], f32, kind="ExternalOutput")
        dbg_y2t = nc.dram_tensor("dbg_y2t", [P, EC * T], bf16, kind="ExternalOutput")
        dbg_h = nc.dram_tensor("dbg_h", [P, FC * T], bf16, kind="ExternalOutput")

    # per-own-head AllToAll bounce buffers
    a2a_in = nc.dram_tensor("a2a_in", [HPC, NCORES, D, T], bf16)
    a2a_out = nc.dram_tensor("a2a_out", [HPC, NCORES, D, T], bf16)
    a2a_groups = [list(range(NCORES))]

    with tile.TileContext(nc) as tc:
        const_pool = tc.alloc_tile_pool(name="const", bufs=1)
        acts = tc.alloc_tile_pool(name="acts", bufs=1)
        stream = tc.alloc_tile_pool(name="stream", bufs=1)

        # ---------------- constants ----------------
        ident = const_pool.tile([P, P], bf16)
        make_identity(nc, ident)
        ones_col = const_pool.tile([P, 1], bf16)
        nc.vector.memset(ones_col, 1.0)
        ones64 = const_pool.tile([1, 64], bf16)
        nc.vector.memset(ones64, 1.0)
        eps_col = const_pool.tile([1, 1], f32)
        nc.vector.memset(eps_col, EPS)
        eps_col2 = const_pool.tile([P, 1], f32)
        nc.vector.memset(eps_col2, EPS)

        ln1g_col = const_pool.tile([P, EC], f32)
        nc.sync.dma_start(ln1g_col, ln1g_in.rearrange("(c p) -> p c", p=P))
        ln1b_col = const_pool.tile([P, EC], f32)
        nc.sync.dma_start(ln1b_col, ln1b_in.rearrange("(c p) -> p c", p=P))
        bqc = const_pool.tile([P, 2], f32)
        nc.sync.dma_start(bqc, bq_in.rearrange("(c p) -> p c", p=P))
        bkc = const_pool.tile([P, 2], f32)
        nc.sync.dma_start(bkc, bk_in.rearrange("(c p) -> p c", p=P))
        b1_col = const_pool.tile([P, FC], f32)
        nc.sync.dma_start(b1_col, b1_in.rearrange("(c p) -> p c", p=P))

        # free-axis rows replicated across partitions
        reps = {}
        for name, t_in, width in [
            ("bv", bv_in, HD), ("bo", bo_in, E), ("b2", b2_in, E),
            ("ln2_g", ln2g_in, E), ("ln2_b", ln2b_in, E),
        ]:
            row = const_pool.tile([1, width], f32, name=f"{name}_row")
            nc.sync.dma_start(row, t_in[None, :])
            rep = const_pool.tile([P, width], f32, name=f"{name}_rep")
            nc.gpsimd.partition_broadcast(rep, row)
            reps[name] = rep

        # ================ phase 1: stats, x-hat, QKV ================
        ph1_sb = tc.alloc_tile_pool(name="ph1_sb", bufs=1)
        ph1a_ps = tc.alloc_tile_pool(name="ph1a_ps", bufs=1, space="PSUM")

        xt = ph1_sb.tile([P, EC, S], bf16)
        xt_v = xT_in.rearrange("(c p) t -> p c t", p=P)
        for ec in range(EC):
            nc.sync.dma_start(xt[:, ec, :], xt_v[:, ec, :])
        xo = acts.tile([P, TC, E], f32)
        nc.sync.dma_start(xo, xo_in.rearrange("(c p) e -> p c e", p=P))

        wq_sb = ph1_sb.tile([P, EC, HD], bf16)
        nc.sync.dma_start(wq_sb, wq_in.rearrange("(c p) d -> p c d", p=P))
        wk_sb = ph1_sb.tile([P, EC, HD], bf16)
        nc.sync.dma_start(wk_sb, wk_in.rearrange("(c p) d -> p c d", p=P))
        wv_sb = ph1_sb.tile([P, EC, HD], bf16)
        nc.sync.dma_start(wv_sb, wv_in.rearrange("(c p) d -> p c d", p=P))

        # LN1 stats for all 2048 tokens: col-sums of x and x^2 via PE
        st_s = [
            ph1a_ps.tile([1, 512], f32, tag=f"sts{qb}", bufs=1, name=f"sts{qb}")
            for qb in range(4)
        ]
        st_q = [
            ph1a_ps.tile([1, 512], f32, tag=f"stq{qb}", bufs=1, name=f"stq{qb}")
            for qb in range(4)
        ]
        for ec in range(EC):
            for qb in range(4):
                sl = slice(qb * 512, (qb + 1) * 512)
                nc.tensor.matmul(
                    st_s[qb], ones_col, xt[:, ec, sl],
                    start=(ec == 0), stop=(ec == EC - 1),
                )
        for ec in range(EC):
            sq = stream.tile([P, S], bf16, tag="sq", bufs=2, name="sq")
            nc.scalar.activation(sq, xt[:, ec, :], Act.Square)
            for qb in range(4):
                sl = slice(qb * 512, (qb + 1) * 512)
                nc.tensor.matmul(
                    st_q[qb], ones_col, sq[:, sl],
                    start=(ec == 0), stop=(ec == EC - 1),
                )

        rs_b = ph1_sb.tile([P, S], bf16)
        murs_b = ph1_sb.tile([P, S], bf16)
        for qb in range(4):
            sl = slice(qb * 512, (qb + 1) * 512)
            mean = ph1_sb.tile([1, 512], f32, name=f"mean{qb}")
            nc.vector.tensor_scalar(mean, st_s[qb], 1.0 / E, None, Alu.mult)
            var = ph1_sb.tile([1, 512], f32, name=f"var{qb}")
            nc.vector.tensor_scalar(var, st_q[qb], 1.0 / E, None, Alu.mult)
            msq = ph1_sb.tile([1, 512], f32, name=f"msq{qb}")
            nc.vector.tensor_tensor(msq, mean, mean, Alu.mult)
            nc.vector.tensor_tensor(var, var, msq, Alu.subtract)
            lnv = ph1_sb.tile([1, 512], f32, name=f"lnv{qb}")
            nc.scalar.activation(lnv, var, Act.Ln, bias=eps_col)
            rsq = ph1_sb.tile([1, 512], f32, name=f"rsq{qb}")
            nc.scalar.activation(rsq, lnv, Act.Exp, scale=-0.5)
            rs_bf = ph1_sb.tile([1, 512], bf16, name=f"rsbf{qb}")
            nc.vector.tensor_copy(rs_bf, rsq)
            murs_bf = ph1_sb.tile([1, 512], bf16, name=f"mursbf{qb}")
            nc.vector.tensor_tensor(murs_bf, mean, rsq, Alu.mult)
            nc.gpsimd.partition_broadcast(rs_b[:, sl], rs_bf)
            nc.gpsimd.partition_broadcast(murs_b[:, sl], murs_bf)

        # x-hat^T = ((x*rs) - mu*rs) * g + b   (bf16, in-place over xt)
        xhat = xt
        for ec in range(EC):
            t1 = stream.tile([P, S], bf16, tag="xh1", bufs=2, name="xh1")
            nc.vector.tensor_tensor(t1, xt[:, ec, :], rs_b, Alu.mult)
            nc.vector.tensor_tensor(t1, t1, murs_b, Alu.subtract)
            nc.vector.tensor_scalar(
                xhat[:, ec, :], t1,
                ln1g_col[:, ec, None], ln1b_col[:, ec, None],
                Alu.mult, Alu.add,
            )

        ph1a_ps.release()
        ph1b_ps = tc.alloc_tile_pool(name="ph1b_ps", bufs=1, space="PSUM")

        # K^T and Q^T for own heads over all tokens: [HD rows, S]
        kT_a = acts.tile([P, S], bf16)
        kT_b = acts.tile([64, S], bf16)
        qT_a = acts.tile([P, S], bf16)
        qT_b = acts.tile([64, S], bf16)
        for (w_sb, bc_col, dst_a, dst_b) in (
            (wk_sb, bkc, kT_a, kT_b),
            (wq_sb, bqc, qT_a, qT_b),
        ):
            for qb in range(4):
                sl = slice(qb * 512, (qb + 1) * 512)
                psa = ph1b_ps.tile([P, 512], f32, tag="proj", bufs=2, name="proj")
                psb = ph1b_ps.tile([64, 512], f32, tag="projB", bufs=2, name="projB")
                for ec in range(EC):
                    nc.tensor.matmul(
                        psa, w_sb[:, ec, 0:P], xhat[:, ec, sl],
                        start=(ec == 0), stop=(ec == EC - 1),
                    )
                    nc.tensor.matmul(
                        psb, w_sb[:, ec, P:HD], xhat[:, ec, sl],
                        start=(ec == 0), stop=(ec == EC - 1),
                    )
                nc.vector.tensor_scalar(
                    dst_a[:, sl], psa, bc_col[:, 0, None], None, Alu.add
                )
                nc.vector.tensor_scalar(
                    dst_b[:, sl], psb, bc_col[0:64, 1, None], None, Alu.add
                )

        # V natural (per key chunk), ones-augmented: [128k, KC, HPC, D+1]
        v3 = acts.tile([P, KC, HPC, D + 1], bf16)
        for kc in range(KC):
            for i in range(HPC):
                nc.vector.memset(v3[:, kc, i, D, None], 1.0)
        for kc in range(KC):
            vp = ph1b_ps.tile([P, HD], f32, tag="vp", bufs=2, name="vp")
            tsl = slice(kc * P, (kc + 1) * P)
            for ec in range(EC):
                nc.tensor.matmul(
                    vp, xhat[:, ec, tsl], wv_sb[:, ec, :],
                    start=(ec == 0), stop=(ec == EC - 1),
                )
            for i in range(HPC):
                nc.vector.tensor_tensor(
                    v3[:, kc, i, 0:D], vp[:, i * D:(i + 1) * D],
                    reps["bv"][:, i * D:(i + 1) * D], Alu.add,
                )

        if DBG:
            nc.sync.dma_start(dbg_rs[:, :], rs_b)
            nc.sync.dma_start(dbg_mu[:, :], murs_b)
            nc.sync.dma_start(dbg_xh[:, :], xhat[:, 0, :])
            nc.sync.dma_start(dbg_kt[:, :], kT_a)
            nc.sync.dma_start(dbg_v3[:, :], v3.rearrange("p a b c -> p (a b c)"))

        ph1_sb.release()
        ph1b_ps.release()

        # ================ phase 2: attention (3 own heads) ================
        att_sb = tc.alloc_tile_pool(name="att_sb", bufs=1)
        att_ps = tc.alloc_tile_pool(name="att_ps", bufs=1, space="PSUM")

        # prefetch heavy phase-3 weights early (overlaps attention)
        wop_sb = att_sb.tile([P, 2 * EC, E], bf16)
        nc.sync.dma_start(wop_sb, wop_in.rearrange("(c p) o -> p c o", p=P))

        for i in range(HPC):
            if i == 0:
                krows, qrows = kT_a[0:64], qT_a[0:64]
            elif i == 1:
                krows, qrows = kT_a[64:128], qT_a[64:128]
            else:
                krows, qrows = kT_b[0:64], qT_b[0:64]
            ctxT = att_sb.tile([64, S], bf16, tag="ctxT", bufs=2, name="ctxT")
            for qb in range(QB):
                ctx_ps = att_ps.tile([D + 1, QW], f32, tag="ctx", bufs=1, name="ctx")
                exps = [None] * KC

                def emit_av(kc):
                    for h2 in range(2):
                        nc.tensor.matmul(
                            ctx_ps[:, h2 * 512:(h2 + 1) * 512],
                            v3[:, kc, i, :],
                            exps[kc][:, h2 * 512:(h2 + 1) * 512],
                            start=(kc == 0), stop=(kc == KC - 1),
                        )

                for kc in range(KC):
                    s_ps = att_ps.tile([P, QW], f32, tag="sps", bufs=3, name="sps")
                    for h2 in range(2):
                        nc.tensor.matmul(
                            s_ps[:, h2 * 512:(h2 + 1) * 512],
                            krows[:, kc * P:(kc + 1) * P],
                            qrows[:, qb * QW + h2 * 512: qb * QW + (h2 + 1) * 512],
                            start=True, stop=True,
                        )
                    exps[kc] = att_sb.tile([P, QW], bf16, tag="exp", bufs=3, name="exp")
                    nc.scalar.activation(exps[kc], s_ps, Act.Exp, scale=SCALE)
                    if kc >= 1:
                        emit_av(kc - 1)
                emit_av(KC - 1)
                den = att_sb.tile([1, QW], f32, tag="den", bufs=2, name="den")
                nc.vector.tensor_copy(den, ctx_ps[D:D + 1, :])
                den_f = att_sb.tile([1, QW], f32, tag="denf", bufs=2, name="denf")
                nc.vector.reciprocal_approx_fast(den_f, den)
                bc_rep = att_sb.tile([64, QW], f32, tag="bcr", bufs=2, name="bcr")
                nc.gpsimd.partition_broadcast(bc_rep, den_f)
                nc.vector.tensor_tensor(
                    ctxT[:, qb * QW:(qb + 1) * QW], ctx_ps[0:64, :], bc_rep, Alu.mult
                )
            if DBG:
                nc.sync.dma_start(dbg_ctx[i], ctxT)
            for j in range(NCORES):
                r = j % TPG
                nc.sync.dma_start(a2a_in[i, j], ctxT[:, r * T:(r + 1) * T])
            nc.gpsimd.collective_compute(
                "AllToAll", mybir.AluOpType.bypass,
                replica_groups=a2a_groups,
                ins=[a2a_in[i]],
                outs=[a2a_out[i]],
            )

        # ================ phase 3: assemble ctx, out-proj, residual =======
        # frame row j*HD + i*D + d  <->  a2a_out[i, j, d, :]
        ctx_all = att_sb.tile([P, 2 * EC, T], bf16)
        for j in range(NCORES):
            for i in range(HPC):
                row = j * HD + i * D
                cc, po = row // P, row % P
                nc.sync.dma_start(ctx_all[po:po + D, cc, :], a2a_out[i, j])

        if DBG:
            nc.sync.dma_start(dbg_ca[:, :], ctx_all.rearrange("p a b -> p (a b)"))
            for j in range(NCORES):
                bnc_i = stream.tile([64, T], bf16, tag="bnci", bufs=1, name="bnci")
                nc.sync.dma_start(bnc_i, a2a_in[0, j])
                nc.sync.dma_start(dbg_a2i[j], bnc_i)
                bnc_o = stream.tile([64, T], bf16, tag="bnco", bufs=1, name="bnco")
                nc.sync.dma_start(bnc_o, a2a_out[0, j])
                nc.sync.dma_start(dbg_a2o[j], bnc_o)
        att_ps.release()
        ph3_ps = tc.alloc_tile_pool(name="ph3_ps", bufs=1, space="PSUM")

        y_sb = acts.tile([P, TC, E], f32)
        for c in range(TC):
            tsl = slice(c * P, (c + 1) * P)
            for off, wdt in ((0, 512), (512, 256)):
                ps = ph3_ps.tile([P, 512], f32, tag="zo", bufs=2, name="zo")[:, :wdt]
                cc_order = [0, 3, 6, 9, 1, 2, 4, 5, 7, 8, 10, 11]
                for n_cc, cc in enumerate(cc_order):
                    nc.tensor.matmul(
                        ps, ctx_all[:, cc, tsl], wop_sb[:, cc, off:off + wdt],
                        start=(n_cc == 0), stop=(n_cc == 2 * EC - 1),
                    )
                osl = slice(off, off + wdt)
                nc.vector.tensor_tensor(
                    y_sb[:, c, osl], ps, reps["bo"][:, osl], Alu.add
                )
                nc.vector.tensor_tensor(
                    y_sb[:, c, osl], y_sb[:, c, osl], xo[:, c, osl], Alu.add
                )
        att_sb.release()

        if DBG:
            nc.sync.dma_start(dbg_y[:, :], y_sb.rearrange("p a b -> p (a b)"))
        # ================ phase 4: LN2 + transpose ================
        ffn_sb = tc.alloc_tile_pool(name="ffn_sb", bufs=1)
        stats2 = ffn_sb.tile([P, TC, 4], f32)
        s2 = stats2[:, :, 0]
        ss2 = stats2[:, :, 1]
        m2 = stats2[:, :, 2]
        r2 = stats2[:, :, 3]
        y2 = ffn_sb.tile([P, TC, E], bf16)
        y2T = ffn_sb.tile([P, EC, T], bf16)
        var2 = ffn_sb.tile([P, TC], f32)
        msq2 = ffn_sb.tile([P, TC], f32)
        lnv2 = ffn_sb.tile([P, TC], f32)
        for c in range(TC):
            sq2 = stream.tile([P, E], f32, tag="sq2", bufs=2, name="sq2")
            nc.vector.tensor_reduce(s2[:, c, None], y_sb[:, c, :], Axis.X, Alu.add)
            nc.scalar.activation(sq2, y_sb[:, c, :], Act.Square)
            nc.vector.tensor_reduce(ss2[:, c, None], sq2, Axis.X, Alu.add)
            nc.vector.tensor_scalar(m2[:, c, None], s2[:, c, None], 1.0 / E, None, Alu.mult)
            nc.vector.tensor_scalar(var2[:, c, None], ss2[:, c, None], 1.0 / E, None, Alu.mult)
            nc.vector.tensor_tensor(msq2[:, c, None], m2[:, c, None], m2[:, c, None], Alu.mult)
            nc.vector.tensor_tensor(var2[:, c, None], var2[:, c, None], msq2[:, c, None], Alu.subtract)
            nc.scalar.activation(lnv2[:, c, None], var2[:, c, None], Act.Ln, bias=eps_col2)
            nc.scalar.activation(r2[:, c, None], lnv2[:, c, None], Act.Exp, scale=-0.5)
            nc.vector.tensor_scalar(
                y2[:, c, :], y_sb[:, c, :],
                m2[:, c, None], r2[:, c, None],
                Alu.subtract, Alu.mult,
            )
            nc.vector.tensor_tensor(y2[:, c, :], y2[:, c, :], reps["ln2_g"], Alu.mult)
            nc.vector.tensor_tensor(y2[:, c, :], y2[:, c, :], reps["ln2_b"], Alu.add)
            for ec in range(EC):
                tps = ph3_ps.tile([P, P], bf16, tag="tp", bufs=2, name="tp")
                nc.tensor.transpose(tps, y2[:, c, ec * P:(ec + 1) * P], ident)
                nc.vector.tensor_copy(y2T[:, ec, c * P:(c + 1) * P], tps)
        ph3_ps.release()

        if DBG:
            nc.sync.dma_start(dbg_y2t[:, :], y2T.rearrange("p a b -> p (a b)"))
        # ================ phase 5: FFN ================
        ffn_ps = tc.alloc_tile_pool(name="ffn_ps", bufs=1, space="PSUM")
        hT = ffn_sb.tile([P, FC, T], bf16)
        w2_sb = ffn_sb.tile([P, FC, E], bf16)
        nc.sync.dma_start(w2_sb, w2_in.rearrange("(c p) o -> p c o", p=P))
        for fc in range(FC):
            w1b = ffn_sb.tile([P, EC, P], bf16, tag="w1b", bufs=3, name="w1b")
            nc.sync.dma_start(
                w1b, w1_in[:, fc * P:(fc + 1) * P].rearrange("(c p) h -> p c h", p=P)
            )
            hps = ffn_ps.tile([P, T], f32, tag="h", bufs=3, name="h")
            for ec in range(EC):
                nc.tensor.matmul(
                    hps, w1b[:, ec, :], y2T[:, ec, :],
                    start=(ec == 0), stop=(ec == EC - 1),
                )
            nc.scalar.activation(hT[:, fc, :], hps, Act.Gelu, bias=b1_col[:, fc, None])

        if DBG:
            nc.sync.dma_start(dbg_h[:, :], hT.rearrange("p a b -> p (a b)"))
        for c in range(TC):
            tsl = slice(c * P, (c + 1) * P)
            za = ffn_ps.tile([P, 512], f32, tag="zf1", bufs=2, name="zf1")
            zb = ffn_ps.tile([P, 256], f32, tag="zf2", bufs=2, name="zf2")
            for fc in range(FC):
                nc.tensor.matmul(
                    za, hT[:, fc, tsl], w2_sb[:, fc, 0:512],
                    start=(fc == 0), stop=(fc == FC - 1),
                )
                nc.tensor.matmul(
                    zb, hT[:, fc, tsl], w2_sb[:, fc, 512:768],
                    start=(fc == 0), stop=(fc == FC - 1),
                )
            o_sb = stream.tile([P, E], f32, tag="o", bufs=2, name="o")
            nc.vector.tensor_tensor(o_sb[:, 0:512], za, y_sb[:, c, 0:512], Alu.add)
            nc.vector.tensor_tensor(o_sb[:, 512:768], zb, y_sb[:, c, 512:768], Alu.add)
            nc.vector.tensor_tensor(o_sb, o_sb, reps["b2"], Alu.add)
            nc.sync.dma_start(out_dram[c * P:(c + 1) * P, :], o_sb)

        ffn_ps.release()
        ffn_sb.release()
        stream.release()
        acts.release()
        const_pool.release()

    nc.finalize()
    return nc


def _get_nc():
    if "nc" not in _CACHE:
        _CACHE["nc"] = _build_nc()
    return _CACHE["nc"]


def _shard_inputs(inputs):
    import ml_dtypes

    bf16 = ml_dtypes.bfloat16
    x = np.asarray(inputs["x"], dtype=np.float32)
    f = {k: np.asarray(v, dtype=np.float32) for k, v in inputs.items() if k != "x"}

    xT = [np.ascontiguousarray(x[g].T).astype(bf16) for g in range(B)]
    wo = f["wo"]

    in_maps = []
    for c in range(NCORES):
        g, r = c // TPG, c % TPG
        hsl = slice(HD * r, HD * r + HD)

        wop = np.zeros((NCORES * HD, E), np.float32)
        for j in range(NCORES):
            if j // TPG == g:
                wop[j * HD:(j + 1) * HD] = wo[HD * (j % TPG): HD * (j % TPG) + HD]

        def pad(b):
            v = np.zeros(2 * P, np.float32)
            v[:HD] = b
            return v

        m = {
            "xT": xT[g],
            "x_own": np.ascontiguousarray(x[g, r * T:(r + 1) * T]),
            "wq": np.ascontiguousarray(f["wq"][:, hsl]).astype(bf16),
            "wk": np.ascontiguousarray(f["wk"][:, hsl]).astype(bf16),
            "wv": np.ascontiguousarray(f["wv"][:, hsl]).astype(bf16),
            "bq": pad(f["bq"][hsl]),
            "bk": pad(f["bk"][hsl]),
            "bv": np.ascontiguousarray(f["bv"][hsl]),
            "wop": wop.astype(bf16),
            "bo": f["bo"],
            "ln1_g": f["ln1_g"], "ln1_b": f["ln1_b"],
            "ln2_g": f["ln2_g"], "ln2_b": f["ln2_b"],
            "w1": f["w1"].astype(bf16), "b1": f["b1"],
            "w2": f["w2"].astype(bf16), "b2": f["b2"],
        }
        in_maps.append(m)
    return in_maps


def kernel(**inputs):
    from concourse.bass_utils import run_bass_kernel_spmd

    nc = _get_nc()
    in_maps = _shard_inputs(inputs)
    res = run_bass_kernel_spmd(nc, in_maps, core_ids=list(range(NCORES)))
    _CACHE["last_results"] = res
    out = np.empty((B, S, E), np.float32)
    for c in range(NCORES):
        g, r = c // TPG, c % TPG
        out[g, r * T:(r + 1) * T, :] = res.results[c]["out"]
    return out


# revision 26
# speedup vs baseline: 1.1972x; 1.0060x over previous
"""Trainium2 Bass kernel for a dense transformer block (B=2, S=2048, E=768, H=12).

Sharding: 8 cores = 2 batch groups x 4 ranks. Head-parallel attention:
core (g, r) owns heads [3r, 3r+3) of batch element g and token rows
[512r, 512r+512) for everything token-local (residuals, LN2, FFN, output).

The host replicates x^T (bf16) across each batch group, so LN1 stats and
Q/K/V projections for the core's own heads over the FULL sequence start
immediately with no collective. After attention, each core holds ctx for
its 3 heads over all 2048 tokens; a per-head 8-core AllToAll sends each
rank the ctx slice for its own 512 tokens. The receive frame interleaves
both batch groups; the output projection contracts over the full 1536-row
frame with a host-permuted wo whose cross-group rows are zeroed, keeping
the device program SPMD-uniform. FFN is token-parallel with full streamed
weights. All matmul operands are bf16 (fp32 PSUM accumulation); softmax
skips max-subtraction and gets the denominator via a ones-augmented V
column.
"""

import numpy as np

B, S, E, H, D = 2, 2048, 768, 12, 64
F = 4 * E
NCORES = 8
TPG = 4                 # ranks per batch group
T = S // TPG            # 512 own tokens
HPC = H // TPG          # 3 heads per core
HD = HPC * D            # 192 own head dims
P = 128
EC = E // P             # 6 embed chunks
FC = F // P             # 24 ffn-hidden chunks
TC = T // P             # 4 own token chunks
KC = S // P             # 16 key chunks (full seq)
QB = 2                  # query blocks of 1024
QW = S // QB            # 1024
EPS = 1e-5
SCALE = 1.0 / float(np.sqrt(E))

_CACHE = {}


def _build_nc():
    import concourse.bass as bass
    import concourse.mybir as mybir
    import concourse.tile as tile
    from concourse import bacc
    from concourse.masks import make_identity

    dt = mybir.dt
    f32 = dt.float32
    bf16 = dt.bfloat16
    Alu = mybir.AluOpType
    Act = mybir.ActivationFunctionType
    Axis = mybir.AxisListType

    nc = bacc.Bacc(
        "TRN2",
        target_bir_lowering=False,
        debug=False,
        enable_asserts=False,
        num_devices=NCORES,
    )

    xT_in = nc.dram_tensor("xT", [E, S], bf16, kind="ExternalInput")
    xo_in = nc.dram_tensor("x_own", [T, E], f32, kind="ExternalInput")
    wq_in = nc.dram_tensor("wq", [E, HD], bf16, kind="ExternalInput")
    wk_in = nc.dram_tensor("wk", [E, HD], bf16, kind="ExternalInput")
    wv_in = nc.dram_tensor("wv", [E, HD], bf16, kind="ExternalInput")
    bq_in = nc.dram_tensor("bq", [2 * P], f32, kind="ExternalInput")
    bk_in = nc.dram_tensor("bk", [2 * P], f32, kind="ExternalInput")
    bv_in = nc.dram_tensor("bv", [HD], f32, kind="ExternalInput")
    wop_in = nc.dram_tensor("wop", [NCORES * HD, E], bf16, kind="ExternalInput")
    bo_in = nc.dram_tensor("bo", [E], f32, kind="ExternalInput")
    ln1g_in = nc.dram_tensor("ln1_g", [E], f32, kind="ExternalInput")
    ln1b_in = nc.dram_tensor("ln1_b", [E], f32, kind="ExternalInput")
    ln2g_in = nc.dram_tensor("ln2_g", [E], f32, kind="ExternalInput")
    ln2b_in = nc.dram_tensor("ln2_b", [E], f32, kind="ExternalInput")
    w1_in = nc.dram_tensor("w1", [E, F], bf16, kind="ExternalInput")
    b1_in = nc.dram_tensor("b1", [F], f32, kind="ExternalInput")
    w2_in = nc.dram_tensor("w2", [F, E], bf16, kind="ExternalInput")
    b2_in = nc.dram_tensor("b2", [E], f32, kind="ExternalInput")
    out_dram = nc.dram_tensor("out", [T, E], f32, kind="ExternalOutput")
    import os as _os
    DBG = bool(_os.environ.get("KBUILD_DEBUG"))
    if DBG:
        dbg_rs = nc.dram_tensor("dbg_rs", [P, S], bf16, kind="ExternalOutput")
        dbg_mu = nc.dram_tensor("dbg_mu", [P, S], bf16, kind="ExternalOutput")
        dbg_xh = nc.dram_tensor("dbg_xh", [P, S], bf16, kind="ExternalOutput")
        dbg_kt = nc.dram_tensor("dbg_kt", [P, S], bf16, kind="ExternalOutput")
        dbg_v3 = nc.dram_tensor("dbg_v3", [P, KC * HPC * (D + 1)], bf16, kind="ExternalOutput")
        dbg_ctx = nc.dram_tensor("dbg_ctx", [HPC, 64, S], bf16, kind="ExternalOutput")
        dbg_a2i = nc.dram_tensor("dbg_a2i", [NCORES, D, T], bf16, kind="ExternalOutput")
        dbg_a2o = nc.dram_tensor("dbg_a2o", [NCORES, D, T], bf16, kind="ExternalOutput")
        dbg_ca = nc.dram_tensor("dbg_ca", [P, 2 * EC * T], bf16, kind="ExternalOutput")
        dbg_y = nc.dram_tensor("dbg_y", [P, TC * E], f32, kind="ExternalOutput")
        dbg_y2t = nc.dram_tensor("dbg_y2t", [P, EC * T], bf16, kind="ExternalOutput")
        dbg_h = nc.dram_tensor("dbg_h", [P, FC * T], bf16, kind="ExternalOutput")

    # per-own-head AllToAll bounce buffers
    a2a_in = nc.dram_tensor("a2a_in", [HPC, NCORES, D, T], bf16)
    a2a_out = nc.dram_tensor("a2a_out", [HPC, NCORES, D, T], bf16)
    a2a_groups = [list(range(NCORES))]

    with tile.TileContext(nc) as tc:
        const_pool = tc.alloc_tile_pool(name="const", bufs=1)
        acts = tc.alloc_tile_pool(name="acts", bufs=1)
        stream = tc.alloc_tile_pool(name="stream", bufs=1)

        # ---------------- constants ----------------
        ident = const_pool.tile([P, P], bf16)
        make_identity(nc, ident)
        ones_col = const_pool.tile([P, 1], bf16)
        nc.vector.memset(ones_col, 1.0)
        ones64 = const_pool.tile([1, 64], bf16)
        nc.vector.memset(ones64, 1.0)
        eps_col = const_pool.tile([1, 1], f32)
        nc.vector.memset(eps_col, EPS)
        eps_col2 = const_pool.tile([P, 1], f32)
        nc.vector.memset(eps_col2, EPS)

        ln1g_col = const_pool.tile([P, EC], f32)
        nc.sync.dma_start(ln1g_col, ln1g_in.rearrange("(c p) -> p c", p=P))
        ln1b_col = const_pool.tile([P, EC], f32)
        nc.sync.dma_start(ln1b_col, ln1b_in.rearrange("(c p) -> p c", p=P))
        bqc = const_pool.tile([P, 2], f32)
        nc.sync.dma_start(bqc, bq_in.rearrange("(c p) -> p c", p=P))
        bkc = const_pool.tile([P, 2], f32)
        nc.sync.dma_start(bkc, bk_in.rearrange("(c p) -> p c", p=P))
        b1_col = const_pool.tile([P, FC], f32)
        nc.sync.dma_start(b1_col, b1_in.rearrange("(c p) -> p c", p=P))

        # free-axis rows replicated across partitions
        reps = {}
        for name, t_in, width in [
            ("bv", bv_in, HD), ("bo", bo_in, E), ("b2", b2_in, E),
            ("ln2_g", ln2g_in, E), ("ln2_b", ln2b_in, E),
        ]:
            row = const_pool.tile([1, width], f32, name=f"{name}_row")
            nc.sync.dma_start(row, t_in[None, :])
            rep = const_pool.tile([P, width], f32, name=f"{name}_rep")
            nc.gpsimd.partition_broadcast(rep, row)
            reps[name] = rep

        # ================ phase 1: stats, x-hat, QKV ================
        ph1_sb = tc.alloc_tile_pool(name="ph1_sb", bufs=1)
        ph1a_ps = tc.alloc_tile_pool(name="ph1a_ps", bufs=1, space="PSUM")

        xt = ph1_sb.tile([P, EC, S], bf16)
        xt_v = xT_in.rearrange("(c p) t -> p c t", p=P)
        for ec in range(EC):
            nc.sync.dma_start(xt[:, ec, :], xt_v[:, ec, :])
        xo = acts.tile([P, TC, E], f32)
        nc.sync.dma_start(xo, xo_in.rearrange("(c p) e -> p c e", p=P))

        wq_sb = ph1_sb.tile([P, EC, HD], bf16)
        nc.sync.dma_start(wq_sb, wq_in.rearrange("(c p) d -> p c d", p=P))
        wk_sb = ph1_sb.tile([P, EC, HD], bf16)
        nc.sync.dma_start(wk_sb, wk_in.rearrange("(c p) d -> p c d", p=P))
        wv_sb = ph1_sb.tile([P, EC, HD], bf16)
        nc.sync.dma_start(wv_sb, wv_in.rearrange("(c p) d -> p c d", p=P))

        # LN1 stats for all 2048 tokens: col-sums of x and x^2 via PE
        st_s = [
            ph1a_ps.tile([1, 512], f32, tag=f"sts{qb}", bufs=1, name=f"sts{qb}")
            for qb in range(4)
        ]
        st_q = [
            ph1a_ps.tile([1, 512], f32, tag=f"stq{qb}", bufs=1, name=f"stq{qb}")
            for qb in range(4)
        ]
        for ec in range(EC):
            for qb in range(4):
                sl = slice(qb * 512, (qb + 1) * 512)
                nc.tensor.matmul(
                    st_s[qb], ones_col, xt[:, ec, sl],
                    start=(ec == 0), stop=(ec == EC - 1),
                )
        for ec in range(EC):
            sq = stream.tile([P, S], bf16, tag="sq", bufs=2, name="sq")
            nc.scalar.activation(sq, xt[:, ec, :], Act.Square)
            for qb in range(4):
                sl = slice(qb * 512, (qb + 1) * 512)
                nc.tensor.matmul(
                    st_q[qb], ones_col, sq[:, sl],
                    start=(ec == 0), stop=(ec == EC - 1),
                )

        rs_b = ph1_sb.tile([P, S], bf16)
        murs_b = ph1_sb.tile([P, S], bf16)
        for qb in range(4):
            sl = slice(qb * 512, (qb + 1) * 512)
            mean = ph1_sb.tile([1, 512], f32, name=f"mean{qb}")
            nc.vector.tensor_scalar(mean, st_s[qb], 1.0 / E, None, Alu.mult)
            var = ph1_sb.tile([1, 512], f32, name=f"var{qb}")
            nc.vector.tensor_scalar(var, st_q[qb], 1.0 / E, None, Alu.mult)
            msq = ph1_sb.tile([1, 512], f32, name=f"msq{qb}")
            nc.vector.tensor_tensor(msq, mean, mean, Alu.mult)
            nc.vector.tensor_tensor(var, var, msq, Alu.subtract)
            lnv = ph1_sb.tile([1, 512], f32, name=f"lnv{qb}")
            nc.scalar.activation(lnv, var, Act.Ln, bias=eps_col)
            rsq = ph1_sb.tile([1, 512], f32, name=f"rsq{qb}")
            nc.scalar.activation(rsq, lnv, Act.Exp, scale=-0.5)
            rs_bf = ph1_sb.tile([1, 512], bf16, name=f"rsbf{qb}")
            nc.vector.tensor_copy(rs_bf, rsq)
            murs_bf = ph1_sb.tile([1, 512], bf16, name=f"mursbf{qb}")
            nc.vector.tensor_tensor(murs_bf, mean, rsq, Alu.mult)
            nc.gpsimd.partition_broadcast(rs_b[:, sl], rs_bf)
            nc.gpsimd.partition_broadcast(murs_b[:, sl], murs_bf)

        # x-hat^T = ((x*rs) - mu*rs) * g + b   (bf16, in-place over xt)
        xhat = xt
        for ec in range(EC):
            t1 = stream.tile([P, S], bf16, tag="xh1", bufs=2, name="xh1")
            nc.vector.tensor_tensor(t1, xt[:, ec, :], rs_b, Alu.mult)
            nc.vector.tensor_tensor(t1, t1, murs_b, Alu.subtract)
            nc.vector.tensor_scalar(
                xhat[:, ec, :], t1,
                ln1g_col[:, ec, None], ln1b_col[:, ec, None],
                Alu.mult, Alu.add,
            )

        ph1a_ps.release()
        ph1b_ps = tc.alloc_tile_pool(name="ph1b_ps", bufs=1, space="PSUM")

        # K^T and Q^T for own heads over all tokens: [HD rows, S]
        kT_a = acts.tile([P, S], bf16)
        kT_b = acts.tile([64, S], bf16)
        qT_a = acts.tile([P, S], bf16)
        qT_b = acts.tile([64, S], bf16)
        for (w_sb, bc_col, dst_a, dst_b) in (
            (wk_sb, bkc, kT_a, kT_b),
            (wq_sb, bqc, qT_a, qT_b),
        ):
            for qb in range(4):
                sl = slice(qb * 512, (qb + 1) * 512)
                psa = ph1b_ps.tile([P, 512], f32, tag="proj", bufs=2, name="proj")
                psb = ph1b_ps.tile([64, 512], f32, tag="projB", bufs=2, name="projB")
                for ec in range(EC):
                    nc.tensor.matmul(
                        psa, w_sb[:, ec, 0:P], xhat[:, ec, sl],
                        start=(ec == 0), stop=(ec == EC - 1),
                    )
                    nc.tensor.matmul(
                        psb, w_sb[:, ec, P:HD], xhat[:, ec, sl],
                        start=(ec == 0), stop=(ec == EC - 1),
                    )
                nc.vector.tensor_scalar(
                    dst_a[:, sl], psa, bc_col[:, 0, None], None, Alu.add
                )
                nc.vector.tensor_scalar(
                    dst_b[:, sl], psb, bc_col[0:64, 1, None], None, Alu.add
                )

        # V natural (per key chunk), ones-augmented: [128k, KC, HPC, D+1]
        v3 = acts.tile([P, KC, HPC, D + 1], bf16)
        for kc in range(KC):
            for i in range(HPC):
                nc.vector.memset(v3[:, kc, i, D, None], 1.0)
        for kc in range(KC):
            vp = ph1b_ps.tile([P, HD], f32, tag="vp", bufs=2, name="vp")
            tsl = slice(kc * P, (kc + 1) * P)
            for ec in range(EC):
                nc.tensor.matmul(
                    vp, xhat[:, ec, tsl], wv_sb[:, ec, :],
                    start=(ec == 0), stop=(ec == EC - 1),
                )
            for i in range(HPC):
                nc.vector.tensor_tensor(
                    v3[:, kc, i, 0:D], vp[:, i * D:(i + 1) * D],
                    reps["bv"][:, i * D:(i + 1) * D], Alu.add,
                )

        if DBG:
            nc.sync.dma_start(dbg_rs[:, :], rs_b)
            nc.sync.dma_start(dbg_mu[:, :], murs_b)
            nc.sync.dma_start(dbg_xh[:, :], xhat[:, 0, :])
            nc.sync.dma_start(dbg_kt[:, :], kT_a)
            nc.sync.dma_start(dbg_v3[:, :], v3.rearrange("p a b c -> p (a b c)"))

        ph1_sb.release()
        ph1b_ps.release()

        # ================ phase 2: attention (3 own heads) ================
        att_sb = tc.alloc_tile_pool(name="att_sb", bufs=1)
        att_ps = tc.alloc_tile_pool(name="att_ps", bufs=1, space="PSUM")

        # prefetch heavy phase-3 weights early (overlaps attention)
        wop_sb = att_sb.tile([P, 2 * EC, E], bf16)
        nc.sync.dma_start(wop_sb, wop_in.rearrange("(c p) o -> p c o", p=P))

        for i in range(HPC):
            if i == 0:
                krows, qrows = kT_a[0:64], qT_a[0:64]
            elif i == 1:
                krows, qrows = kT_a[64:128], qT_a[64:128]
            else:
                krows, qrows = kT_b[0:64], qT_b[0:64]
            ctxT = att_sb.tile([64, S], bf16, tag="ctxT", bufs=2, name="ctxT")
            for qb in range(QB):
                ctx_ps = att_ps.tile([D + 1, QW], f32, tag="ctx", bufs=1, name="ctx")
                exps = [None] * KC

                def emit_av(kc):
                    for h2 in range(2):
                        nc.tensor.matmul(
                            ctx_ps[:, h2 * 512:(h2 + 1) * 512],
                            v3[:, kc, i, :],
                            exps[kc][:, h2 * 512:(h2 + 1) * 512],
                            start=(kc == 0), stop=(kc == KC - 1),
                        )

                for kc in range(KC):
                    s_ps = att_ps.tile([P, QW], f32, tag="sps", bufs=3, name="sps")
                    for h2 in range(2):
                        nc.tensor.matmul(
                            s_ps[:, h2 * 512:(h2 + 1) * 512],
                            krows[:, kc * P:(kc + 1) * P],
                            qrows[:, qb * QW + h2 * 512: qb * QW + (h2 + 1) * 512],
                            start=True, stop=True,
                        )
                    exps[kc] = att_sb.tile([P, QW], bf16, tag="exp", bufs=3, name="exp")
                    nc.scalar.activation(exps[kc], s_ps, Act.Exp, scale=SCALE)
                    if kc >= 1:
                        emit_av(kc - 1)
                emit_av(KC - 1)
                den = att_sb.tile([1, QW], f32, tag="den", bufs=2, name="den")
                nc.vector.tensor_copy(den, ctx_ps[D:D + 1, :])
                den_f = att_sb.tile([1, QW], f32, tag="denf", bufs=2, name="denf")
                nc.vector.reciprocal_approx_fast(den_f, den)
                bc_rep = att_sb.tile([64, QW], f32, tag="bcr", bufs=2, name="bcr")
                nc.gpsimd.partition_broadcast(bc_rep, den_f)
                nc.vector.tensor_tensor(
                    ctxT[:, qb * QW:(qb + 1) * QW], ctx_ps[0:64, :], bc_rep, Alu.mult
                )
            if DBG:
                nc.sync.dma_start(dbg_ctx[i], ctxT)
            for j in range(NCORES):
                r = j % TPG
                nc.sync.dma_start(a2a_in[i, j], ctxT[:, r * T:(r + 1) * T])
            nc.gpsimd.collective_compute(
                "AllToAll", mybir.AluOpType.bypass,
                replica_groups=a2a_groups,
                ins=[a2a_in[i]],
                outs=[a2a_out[i]],
            )

        # ================ phase 3: assemble ctx, out-proj, residual =======
        # frame row j*HD + i*D + d  <->  a2a_out[i, j, d, :]
        ctx_all = att_sb.tile([P, 2 * EC, T], bf16)
        for i in range(HPC):
            for j in range(NCORES):
                row = j * HD + i * D
                cc, po = row // P, row % P
                nc.sync.dma_start(ctx_all[po:po + D, cc, :], a2a_out[i, j])

        if DBG:
            nc.sync.dma_start(dbg_ca[:, :], ctx_all.rearrange("p a b -> p (a b)"))
            for j in range(NCORES):
                bnc_i = stream.tile([64, T], bf16, tag="bnci", bufs=1, name="bnci")
                nc.sync.dma_start(bnc_i, a2a_in[0, j])
                nc.sync.dma_start(dbg_a2i[j], bnc_i)
                bnc_o = stream.tile([64, T], bf16, tag="bnco", bufs=1, name="bnco")
                nc.sync.dma_start(bnc_o, a2a_out[0, j])
                nc.sync.dma_start(dbg_a2o[j], bnc_o)
        att_ps.release()
        ph3_ps = tc.alloc_tile_pool(name="ph3_ps", bufs=1, space="PSUM")

        y_sb = acts.tile([P, TC, E], f32)
        for c in range(TC):
            tsl = slice(c * P, (c + 1) * P)
            for off, wdt in ((0, 512), (512, 256)):
                ps = ph3_ps.tile([P, 512], f32, tag="zo", bufs=2, name="zo")[:, :wdt]
                cc_order = [0, 3, 6, 9, 1, 2, 4, 5, 7, 8, 10, 11]
                for n_cc, cc in enumerate(cc_order):
                    nc.tensor.matmul(
                        ps, ctx_all[:, cc, tsl], wop_sb[:, cc, off:off + wdt],
                        start=(n_cc == 0), stop=(n_cc == 2 * EC - 1),
                    )
                osl = slice(off, off + wdt)
                nc.vector.tensor_tensor(
                    y_sb[:, c, osl], ps, reps["bo"][:, osl], Alu.add
                )
                nc.vector.tensor_tensor(
                    y_sb[:, c, osl], y_sb[:, c, osl], xo[:, c, osl], Alu.add
                )
        att_sb.release()

        if DBG:
            nc.sync.dma_start(dbg_y[:, :], y_sb.rearrange("p a b -> p (a b)"))
        # ================ phase 4: LN2 + transpose ================
        ffn_sb = tc.alloc_tile_pool(name="ffn_sb", bufs=1)
        stats2 = ffn_sb.tile([P, TC, 4], f32)
        s2 = stats2[:, :, 0]
        ss2 = stats2[:, :, 1]
        m2 = stats2[:, :, 2]
        r2 = stats2[:, :, 3]
        y2 = ffn_sb.tile([P, TC, E], bf16)
        y2T = ffn_sb.tile([P, EC, T], bf16)
        var2 = ffn_sb.tile([P, TC], f32)
        msq2 = ffn_sb.tile([P, TC], f32)
        lnv2 = ffn_sb.tile([P, TC], f32)
        for c in range(TC):
            sq2 = stream.tile([P, E], f32, tag="sq2", bufs=2, name="sq2")
            nc.vector.tensor_reduce(s2[:, c, None], y_sb[:, c, :], Axis.X, Alu.add)
            nc.scalar.activation(sq2, y_sb[:, c, :], Act.Square)
            nc.vector.tensor_reduce(ss2[:, c, None], sq2, Axis.X, Alu.add)
            nc.vector.tensor_scalar(m2[:, c, None], s2[:, c, None], 1.0 / E, None, Alu.mult)
            nc.vector.tensor_scalar(var2[:, c, None], ss2[:, c, None], 1.0 / E, None, Alu.mult)
            nc.vector.tensor_tensor(msq2[:, c, None], m2[:, c, None], m2[:, c, None], Alu.mult)
            nc.vector.tensor_tensor(var2[:, c, None], var2[:, c, None], msq2[:, c, None], Alu.subtract)
            nc.scalar.activation(lnv2[:, c, None], var2[:, c, None], Act.Ln, bias=eps_col2)
            nc.scalar.activation(r2[:, c, None], lnv2[:, c, None], Act.Exp, scale=-0.5)
            nc.vector.tensor_scalar(
                y2[:, c, :], y_sb[:, c, :],
                m2[:, c, None], r2[:, c, None],
                Alu.subtract, Alu.mult,
            )
            nc.vector.tensor_tensor(y2[:, c, :], y2[:, c, :], reps["ln2_g"], Alu.mult)
            nc.vector.tensor_tensor(y2[:, c, :], y2[:, c, :], reps["ln2_b"], Alu.add)
            for ec in range(EC):
                tps = ph3_ps.tile([P, P], bf16, tag="tp", bufs=2, name="tp")
                nc.tensor.transpose(tps, y2[:, c, ec * P:(ec + 1) * P], ident)
                nc.vector.tensor_copy(y2T[:, ec, c * P:(c + 1) * P], tps)
        ph3_ps.release()

        if DBG:
            nc.sync.dma_start(dbg_y2t[:, :], y2T.rearrange("p a b -> p (a b)"))
        # ================ phase 5: FFN ================
        ffn_ps = tc.alloc_tile_pool(name="ffn_ps", bufs=1, space="PSUM")
        hT = ffn_sb.tile([P, FC, T], bf16)
        w2_sb = ffn_sb.tile([P, FC, E], bf16)
        nc.sync.dma_start(w2_sb, w2_in.rearrange("(c p) o -> p c o", p=P))
        for fc in range(FC):
            w1b = ffn_sb.tile([P, EC, P], bf16, tag="w1b", bufs=3, name="w1b")
            nc.sync.dma_start(
                w1b, w1_in[:, fc * P:(fc + 1) * P].rearrange("(c p) h -> p c h", p=P)
            )
            hps = ffn_ps.tile([P, T], f32, tag="h", bufs=3, name="h")
            for ec in range(EC):
                nc.tensor.matmul(
                    hps, w1b[:, ec, :], y2T[:, ec, :],
                    start=(ec == 0), stop=(ec == EC - 1),
                )
            nc.scalar.activation(hT[:, fc, :], hps, Act.Gelu, bias=b1_col[:, fc, None])

        if DBG:
            nc.sync.dma_start(dbg_h[:, :], hT.rearrange("p a b -> p (a b)"))
        for c in range(TC):
            tsl = slice(c * P, (c + 1) * P)
            za = ffn_ps.tile([P, 512], f32, tag="zf1", bufs=2, name="zf1")
            zb = ffn_ps.tile([P, 256], f32, tag="zf2", bufs=2, name="zf2")
            for fc in range(FC):
                nc.tensor.matmul(
                    za, hT[:, fc, tsl], w2_sb[:, fc, 0:512],
                    start=(fc == 0), stop=(fc == FC - 1),
                )
                nc.tensor.matmul(
                    zb, hT[:, fc, tsl], w2_sb[:, fc, 512:768],
                    start=(fc == 0), stop=(fc == FC - 1),
                )
            o_sb = stream.tile([P, E], f32, tag="o", bufs=2, name="o")
            nc.vector.tensor_tensor(o_sb[:, 0:512], za, y_sb[:, c, 0:512], Alu.add)
            nc.vector.tensor_tensor(o_sb[:, 512:768], zb, y_sb[:, c, 512:768], Alu.add)
            nc.vector.tensor_tensor(o_sb, o_sb, reps["b2"], Alu.add)
            nc.sync.dma_start(out_dram[c * P:(c + 1) * P, :], o_sb)

        ffn_ps.release()
        ffn_sb.release()
        stream.release()
        acts.release()
        const_pool.release()

    nc.finalize()
    return nc


def _get_nc():
    if "nc" not in _CACHE:
        _CACHE["nc"] = _build_nc()
    return _CACHE["nc"]


def _shard_inputs(inputs):
    import ml_dtypes

    bf16 = ml_dtypes.bfloat16
    x = np.asarray(inputs["x"], dtype=np.float32)
    f = {k: np.asarray(v, dtype=np.float32) for k, v in inputs.items() if k != "x"}

    xT = [np.ascontiguousarray(x[g].T).astype(bf16) for g in range(B)]
    wo = f["wo"]

    in_maps = []
    for c in range(NCORES):
        g, r = c // TPG, c % TPG
        hsl = slice(HD * r, HD * r + HD)

        wop = np.zeros((NCORES * HD, E), np.float32)
        for j in range(NCORES):
            if j // TPG == g:
                wop[j * HD:(j + 1) * HD] = wo[HD * (j % TPG): HD * (j % TPG) + HD]

        def pad(b):
            v = np.zeros(2 * P, np.float32)
            v[:HD] = b
            return v

        m = {
            "xT": xT[g],
            "x_own": np.ascontiguousarray(x[g, r * T:(r + 1) * T]),
            "wq": np.ascontiguousarray(f["wq"][:, hsl]).astype(bf16),
            "wk": np.ascontiguousarray(f["wk"][:, hsl]).astype(bf16),
            "wv": np.ascontiguousarray(f["wv"][:, hsl]).astype(bf16),
            "bq": pad(f["bq"][hsl]),
            "bk": pad(f["bk"][hsl]),
            "bv": np.ascontiguousarray(f["bv"][hsl]),
            "wop": wop.astype(bf16),
            "bo": f["bo"],
            "ln1_g": f["ln1_g"], "ln1_b": f["ln1_b"],
            "ln2_g": f["ln2_g"], "ln2_b": f["ln2_b"],
            "w1": f["w1"].astype(bf16), "b1": f["b1"],
            "w2": f["w2"].astype(bf16), "b2": f["b2"],
        }
        in_maps.append(m)
    return in_maps


def kernel(**inputs):
    from concourse.bass_utils import run_bass_kernel_spmd

    nc = _get_nc()
    in_maps = _shard_inputs(inputs)
    res = run_bass_kernel_spmd(nc, in_maps, core_ids=list(range(NCORES)))
    _CACHE["last_results"] = res
    out = np.empty((B, S, E), np.float32)
    for c in range(NCORES):
        g, r = c // TPG, c % TPG
        out[g, r * T:(r + 1) * T, :] = res.results[c]["out"]
    return out


# revision 27
# speedup vs baseline: 1.2122x; 1.0125x over previous
"""Trainium2 Bass kernel for a dense transformer block (B=2, S=2048, E=768, H=12).

Sharding: 8 cores = 2 batch groups x 4 ranks. Head-parallel attention:
core (g, r) owns heads [3r, 3r+3) of batch element g and token rows
[512r, 512r+512) for everything token-local (residuals, LN2, FFN, output).

The host replicates x^T (bf16) across each batch group, so LN1 stats and
Q/K/V projections for the core's own heads over the FULL sequence start
immediately with no collective. After attention, each core holds ctx for
its 3 heads over all 2048 tokens; a per-head 8-core AllToAll sends each
rank the ctx slice for its own 512 tokens. The receive frame interleaves
both batch groups; the output projection contracts over the full 1536-row
frame with a host-permuted wo whose cross-group rows are zeroed, keeping
the device program SPMD-uniform. FFN is token-parallel with full streamed
weights. All matmul operands are bf16 (fp32 PSUM accumulation); softmax
skips max-subtraction and gets the denominator via a ones-augmented V
column.
"""

import numpy as np

B, S, E, H, D = 2, 2048, 768, 12, 64
F = 4 * E
NCORES = 8
TPG = 4                 # ranks per batch group
T = S // TPG            # 512 own tokens
HPC = H // TPG          # 3 heads per core
HD = HPC * D            # 192 own head dims
P = 128
EC = E // P             # 6 embed chunks
FC = F // P             # 24 ffn-hidden chunks
TC = T // P             # 4 own token chunks
KC = S // P             # 16 key chunks (full seq)
QB = 2                  # query blocks of 1024
QW = S // QB            # 1024
EPS = 1e-5
SCALE = 1.0 / float(np.sqrt(E))

_CACHE = {}


def _build_nc():
    import concourse.bass as bass
    import concourse.mybir as mybir
    import concourse.tile as tile
    from concourse import bacc
    from concourse.masks import make_identity

    dt = mybir.dt
    f32 = dt.float32
    bf16 = dt.bfloat16
    Alu = mybir.AluOpType
    Act = mybir.ActivationFunctionType
    Axis = mybir.AxisListType

    nc = bacc.Bacc(
        "TRN2",
        target_bir_lowering=False,
        debug=False,
        enable_asserts=False,
        num_devices=NCORES,
    )

    xT_in = nc.dram_tensor("xT", [E, S], bf16, kind="ExternalInput")
    xo_in = nc.dram_tensor("x_own", [T, E], f32, kind="ExternalInput")
    wq_in = nc.dram_tensor("wq", [E, HD], bf16, kind="ExternalInput")
    wk_in = nc.dram_tensor("wk", [E, HD], bf16, kind="ExternalInput")
    wv_in = nc.dram_tensor("wv", [E, HD], bf16, kind="ExternalInput")
    bq_in = nc.dram_tensor("bq", [2 * P], f32, kind="ExternalInput")
    bk_in = nc.dram_tensor("bk", [2 * P], f32, kind="ExternalInput")
    bv_in = nc.dram_tensor("bv", [HD], f32, kind="ExternalInput")
    wop_in = nc.dram_tensor("wop", [NCORES * HD, E], bf16, kind="ExternalInput")
    bo_in = nc.dram_tensor("bo", [E], f32, kind="ExternalInput")
    ln1g_in = nc.dram_tensor("ln1_g", [E], f32, kind="ExternalInput")
    ln1b_in = nc.dram_tensor("ln1_b", [E], f32, kind="ExternalInput")
    ln2g_in = nc.dram_tensor("ln2_g", [E], f32, kind="ExternalInput")
    ln2b_in = nc.dram_tensor("ln2_b", [E], f32, kind="ExternalInput")
    w1_in = nc.dram_tensor("w1", [E, F], bf16, kind="ExternalInput")
    b1_in = nc.dram_tensor("b1", [F], f32, kind="ExternalInput")
    w2_in = nc.dram_tensor("w2", [F, E], bf16, kind="ExternalInput")
    b2_in = nc.dram_tensor("b2", [E], f32, kind="ExternalInput")
    out_dram = nc.dram_tensor("out", [T, E], f32, kind="ExternalOutput")
    import os as _os
    DBG = bool(_os.environ.get("KBUILD_DEBUG"))
    if DBG:
        dbg_rs = nc.dram_tensor("dbg_rs", [P, S], bf16, kind="ExternalOutput")
        dbg_mu = nc.dram_tensor("dbg_mu", [P, S], bf16, kind="ExternalOutput")
        dbg_xh = nc.dram_tensor("dbg_xh", [P, S], bf16, kind="ExternalOutput")
        dbg_kt = nc.dram_tensor("dbg_kt", [P, S], bf16, kind="ExternalOutput")
        dbg_v3 = nc.dram_tensor("dbg_v3", [P, KC * HPC * (D + 1)], bf16, kind="ExternalOutput")
        dbg_ctx = nc.dram_tensor("dbg_ctx", [HPC, 64, S], bf16, kind="ExternalOutput")
        dbg_a2i = nc.dram_tensor("dbg_a2i", [NCORES, D, T], bf16, kind="ExternalOutput")
        dbg_a2o = nc.dram_tensor("dbg_a2o", [NCORES, D, T], bf16, kind="ExternalOutput")
        dbg_ca = nc.dram_tensor("dbg_ca", [P, 2 * EC * T], bf16, kind="ExternalOutput")
        dbg_y = nc.dram_tensor("dbg_y", [P, TC * E], f32, kind="ExternalOutput")
        dbg_y2t = nc.dram_tensor("dbg_y2t", [P, EC * T], bf16, kind="ExternalOutput")
        dbg_h = nc.dram_tensor("dbg_h", [P, FC * T], bf16, kind="ExternalOutput")

    # per-own-head AllToAll bounce buffers
    a2a_in = nc.dram_tensor("a2a_in", [HPC, NCORES, D, T], bf16)
    a2a_out = nc.dram_tensor("a2a_out", [HPC, NCORES, D, T], bf16)
    a2a_groups = [list(range(NCORES))]

    with tile.TileContext(nc) as tc:
        const_pool = tc.alloc_tile_pool(name="const", bufs=1)
        acts = tc.alloc_tile_pool(name="acts", bufs=1)
        stream = tc.alloc_tile_pool(name="stream", bufs=1)

        # ---------------- constants ----------------
        ident = const_pool.tile([P, P], bf16)
        make_identity(nc, ident)
        ones_col = const_pool.tile([P, 1], bf16)
        nc.vector.memset(ones_col, 1.0)
        ones64 = const_pool.tile([1, 64], bf16)
        nc.vector.memset(ones64, 1.0)
        eps_col = const_pool.tile([1, 1], f32)
        nc.vector.memset(eps_col, EPS)
        eps_col2 = const_pool.tile([P, 1], f32)
        nc.vector.memset(eps_col2, EPS)

        ln1g_col = const_pool.tile([P, EC], f32)
        nc.sync.dma_start(ln1g_col, ln1g_in.rearrange("(c p) -> p c", p=P))
        ln1b_col = const_pool.tile([P, EC], f32)
        nc.sync.dma_start(ln1b_col, ln1b_in.rearrange("(c p) -> p c", p=P))
        bqc = const_pool.tile([P, 2], f32)
        nc.sync.dma_start(bqc, bq_in.rearrange("(c p) -> p c", p=P))
        bkc = const_pool.tile([P, 2], f32)
        nc.sync.dma_start(bkc, bk_in.rearrange("(c p) -> p c", p=P))
        b1_col = const_pool.tile([P, FC], f32)
        nc.sync.dma_start(b1_col, b1_in.rearrange("(c p) -> p c", p=P))

        # free-axis rows replicated across partitions
        reps = {}
        for name, t_in, width in [
            ("bv", bv_in, HD), ("bo", bo_in, E), ("b2", b2_in, E),
            ("ln2_g", ln2g_in, E), ("ln2_b", ln2b_in, E),
        ]:
            row = const_pool.tile([1, width], f32, name=f"{name}_row")
            nc.sync.dma_start(row, t_in[None, :])
            rep = const_pool.tile([P, width], f32, name=f"{name}_rep")
            nc.gpsimd.partition_broadcast(rep, row)
            reps[name] = rep

        # ================ phase 1: stats, x-hat, QKV ================
        ph1_sb = tc.alloc_tile_pool(name="ph1_sb", bufs=1)
        ph1a_ps = tc.alloc_tile_pool(name="ph1a_ps", bufs=1, space="PSUM")

        xt = ph1_sb.tile([P, EC, S], bf16)
        xt_v = xT_in.rearrange("(c p) t -> p c t", p=P)
        for ec in range(EC):
            nc.sync.dma_start(xt[:, ec, :], xt_v[:, ec, :])
        xo = acts.tile([P, TC, E], f32)
        nc.sync.dma_start(xo, xo_in.rearrange("(c p) e -> p c e", p=P))

        wq_sb = ph1_sb.tile([P, EC, HD], bf16)
        nc.sync.dma_start(wq_sb, wq_in.rearrange("(c p) d -> p c d", p=P))
        wk_sb = ph1_sb.tile([P, EC, HD], bf16)
        nc.sync.dma_start(wk_sb, wk_in.rearrange("(c p) d -> p c d", p=P))
        wv_sb = ph1_sb.tile([P, EC, HD], bf16)
        nc.sync.dma_start(wv_sb, wv_in.rearrange("(c p) d -> p c d", p=P))

        # LN1 stats for all 2048 tokens: col-sums of x and x^2 via PE
        st_s = [
            ph1a_ps.tile([1, 512], f32, tag=f"sts{qb}", bufs=1, name=f"sts{qb}")
            for qb in range(4)
        ]
        st_q = [
            ph1a_ps.tile([1, 512], f32, tag=f"stq{qb}", bufs=1, name=f"stq{qb}")
            for qb in range(4)
        ]
        for ec in range(EC):
            for qb in range(4):
                sl = slice(qb * 512, (qb + 1) * 512)
                nc.tensor.matmul(
                    st_s[qb], ones_col, xt[:, ec, sl],
                    start=(ec == 0), stop=(ec == EC - 1),
                )
        for ec in range(EC):
            sq = stream.tile([P, S], bf16, tag="sq", bufs=2, name="sq")
            nc.scalar.activation(sq, xt[:, ec, :], Act.Square)
            for qb in range(4):
                sl = slice(qb * 512, (qb + 1) * 512)
                nc.tensor.matmul(
                    st_q[qb], ones_col, sq[:, sl],
                    start=(ec == 0), stop=(ec == EC - 1),
                )

        rs_b = ph1_sb.tile([P, S], bf16)
        murs_b = ph1_sb.tile([P, S], bf16)
        for qb in range(4):
            sl = slice(qb * 512, (qb + 1) * 512)
            mean = ph1_sb.tile([1, 512], f32, name=f"mean{qb}")
            nc.vector.tensor_scalar(mean, st_s[qb], 1.0 / E, None, Alu.mult)
            var = ph1_sb.tile([1, 512], f32, name=f"var{qb}")
            nc.vector.tensor_scalar(var, st_q[qb], 1.0 / E, None, Alu.mult)
            msq = ph1_sb.tile([1, 512], f32, name=f"msq{qb}")
            nc.vector.tensor_tensor(msq, mean, mean, Alu.mult)
            nc.vector.tensor_tensor(var, var, msq, Alu.subtract)
            lnv = ph1_sb.tile([1, 512], f32, name=f"lnv{qb}")
            nc.scalar.activation(lnv, var, Act.Ln, bias=eps_col)
            rsq = ph1_sb.tile([1, 512], f32, name=f"rsq{qb}")
            nc.scalar.activation(rsq, lnv, Act.Exp, scale=-0.5)
            rs_bf = ph1_sb.tile([1, 512], bf16, name=f"rsbf{qb}")
            nc.vector.tensor_copy(rs_bf, rsq)
            murs_bf = ph1_sb.tile([1, 512], bf16, name=f"mursbf{qb}")
            nc.vector.tensor_tensor(murs_bf, mean, rsq, Alu.mult)
            nc.gpsimd.partition_broadcast(rs_b[:, sl], rs_bf)
            nc.gpsimd.partition_broadcast(murs_b[:, sl], murs_bf)

        # x-hat^T = ((x*rs) - mu*rs) * g + b   (bf16, in-place over xt)
        xhat = xt
        for ec in range(EC):
            t1 = stream.tile([P, S], bf16, tag="xh1", bufs=2, name="xh1")
            nc.vector.tensor_tensor(t1, xt[:, ec, :], rs_b, Alu.mult)
            nc.vector.tensor_tensor(t1, t1, murs_b, Alu.subtract)
            nc.vector.tensor_scalar(
                xhat[:, ec, :], t1,
                ln1g_col[:, ec, None], ln1b_col[:, ec, None],
                Alu.mult, Alu.add,
            )

        ph1a_ps.release()
        ph1b_ps = tc.alloc_tile_pool(name="ph1b_ps", bufs=1, space="PSUM")

        # K^T and Q^T for own heads over all tokens: [HD rows, S]
        kT_a = acts.tile([P, S], bf16)
        kT_b = acts.tile([64, S], bf16)
        qT_a = acts.tile([P, S], bf16)
        qT_b = acts.tile([64, S], bf16)
        for (w_sb, bc_col, dst_a, dst_b) in (
            (wk_sb, bkc, kT_a, kT_b),
            (wq_sb, bqc, qT_a, qT_b),
        ):
            for qb in range(4):
                sl = slice(qb * 512, (qb + 1) * 512)
                psa = ph1b_ps.tile([P, 512], f32, tag="proj", bufs=2, name="proj")
                psb = ph1b_ps.tile([64, 512], f32, tag="projB", bufs=2, name="projB")
                for ec in range(EC):
                    nc.tensor.matmul(
                        psa, w_sb[:, ec, 0:P], xhat[:, ec, sl],
                        start=(ec == 0), stop=(ec == EC - 1),
                    )
                    nc.tensor.matmul(
                        psb, w_sb[:, ec, P:HD], xhat[:, ec, sl],
                        start=(ec == 0), stop=(ec == EC - 1),
                    )
                nc.vector.tensor_scalar(
                    dst_a[:, sl], psa, bc_col[:, 0, None], None, Alu.add
                )
                nc.vector.tensor_scalar(
                    dst_b[:, sl], psb, bc_col[0:64, 1, None], None, Alu.add
                )

        # V natural (per key chunk), ones-augmented: [128k, KC, HPC, D+1]
        v3 = acts.tile([P, KC, HPC, D + 1], bf16)
        for kc in range(KC):
            for i in range(HPC):
                nc.vector.memset(v3[:, kc, i, D, None], 1.0)
        for kc in range(KC):
            vp = ph1b_ps.tile([P, HD], f32, tag="vp", bufs=2, name="vp")
            tsl = slice(kc * P, (kc + 1) * P)
            for ec in range(EC):
                nc.tensor.matmul(
                    vp, xhat[:, ec, tsl], wv_sb[:, ec, :],
                    start=(ec == 0), stop=(ec == EC - 1),
                )
            for i in range(HPC):
                nc.vector.tensor_tensor(
                    v3[:, kc, i, 0:D], vp[:, i * D:(i + 1) * D],
                    reps["bv"][:, i * D:(i + 1) * D], Alu.add,
                )

        if DBG:
            nc.sync.dma_start(dbg_rs[:, :], rs_b)
            nc.sync.dma_start(dbg_mu[:, :], murs_b)
            nc.sync.dma_start(dbg_xh[:, :], xhat[:, 0, :])
            nc.sync.dma_start(dbg_kt[:, :], kT_a)
            nc.sync.dma_start(dbg_v3[:, :], v3.rearrange("p a b c -> p (a b c)"))

        ph1_sb.release()
        ph1b_ps.release()

        # ================ phase 2: attention (3 own heads) ================
        att_sb = tc.alloc_tile_pool(name="att_sb", bufs=1)
        att_ps = tc.alloc_tile_pool(name="att_ps", bufs=1, space="PSUM")

        # prefetch heavy phase-3 weights early (overlaps attention)
        wop_sb = att_sb.tile([P, 2 * EC, E], bf16)
        nc.sync.dma_start(wop_sb, wop_in.rearrange("(c p) o -> p c o", p=P))

        for i in range(HPC):
            if i == 0:
                krows, qrows = kT_a[0:64], qT_a[0:64]
            elif i == 1:
                krows, qrows = kT_a[64:128], qT_a[64:128]
            else:
                krows, qrows = kT_b[0:64], qT_b[0:64]
            ctxT = att_sb.tile([64, S], bf16, tag="ctxT", bufs=2, name="ctxT")
            for qb in range(QB):
                ctx_ps = att_ps.tile([D + 1, QW], f32, tag="ctx", bufs=1, name="ctx")
                exps = [None] * KC

                def emit_av(kc):
                    for h2 in range(2):
                        nc.tensor.matmul(
                            ctx_ps[:, h2 * 512:(h2 + 1) * 512],
                            v3[:, kc, i, :],
                            exps[kc][:, h2 * 512:(h2 + 1) * 512],
                            start=(kc == 0), stop=(kc == KC - 1),
                        )

                for kc in range(KC):
                    s_ps = att_ps.tile([P, QW], f32, tag="sps", bufs=3, name="sps")
                    for h2 in range(2):
                        nc.tensor.matmul(
                            s_ps[:, h2 * 512:(h2 + 1) * 512],
                            krows[:, kc * P:(kc + 1) * P],
                            qrows[:, qb * QW + h2 * 512: qb * QW + (h2 + 1) * 512],
                            start=True, stop=True,
                        )
                    exps[kc] = att_sb.tile([P, QW], bf16, tag="exp", bufs=3, name="exp")
                    nc.scalar.activation(exps[kc], s_ps, Act.Exp, scale=SCALE)
                    if kc >= 1:
                        emit_av(kc - 1)
                emit_av(KC - 1)
                den = att_sb.tile([1, QW], f32, tag="den", bufs=2, name="den")
                nc.vector.tensor_copy(den, ctx_ps[D:D + 1, :])
                den_f = att_sb.tile([1, QW], f32, tag="denf", bufs=2, name="denf")
                nc.vector.reciprocal_approx_fast(den_f, den)
                bc_rep = att_sb.tile([64, QW], f32, tag="bcr", bufs=2, name="bcr")
                nc.gpsimd.partition_broadcast(bc_rep, den_f)
                nc.vector.tensor_tensor(
                    ctxT[:, qb * QW:(qb + 1) * QW], ctx_ps[0:64, :], bc_rep, Alu.mult
                )
            if DBG:
                nc.sync.dma_start(dbg_ctx[i], ctxT)
            for j in range(NCORES):
                r = j % TPG
                nc.sync.dma_start(a2a_in[i, j], ctxT[:, r * T:(r + 1) * T])
            nc.gpsimd.collective_compute(
                "AllToAll", mybir.AluOpType.bypass,
                replica_groups=a2a_groups,
                ins=[a2a_in[i]],
                outs=[a2a_out[i]],
            )

        # ================ phase 3: assemble ctx, out-proj, residual =======
        # frame row j*HD + i*D + d  <->  a2a_out[i, j, d, :]
        ctx_all = att_sb.tile([P, 2 * EC, T], bf16)
        for i in range(HPC):
            for j in range(NCORES):
                row = j * HD + i * D
                cc, po = row // P, row % P
                nc.sync.dma_start(ctx_all[po:po + D, cc, :], a2a_out[i, j])

        if DBG:
            nc.sync.dma_start(dbg_ca[:, :], ctx_all.rearrange("p a b -> p (a b)"))
            for j in range(NCORES):
                bnc_i = stream.tile([64, T], bf16, tag="bnci", bufs=1, name="bnci")
                nc.sync.dma_start(bnc_i, a2a_in[0, j])
                nc.sync.dma_start(dbg_a2i[j], bnc_i)
                bnc_o = stream.tile([64, T], bf16, tag="bnco", bufs=1, name="bnco")
                nc.sync.dma_start(bnc_o, a2a_out[0, j])
                nc.sync.dma_start(dbg_a2o[j], bnc_o)
        y_sb = acts.tile([P, TC, E], f32)
        cc_early = [0, 3, 6, 9]
        cc_late = [1, 2, 4, 5, 7, 8, 10, 11]
        for c in range(TC):
            tsl = slice(c * P, (c + 1) * P)
            for off, wdt in ((0, 512), (512, 256)):
                osl = slice(off, off + wdt)
                ps = att_ps.tile([P, QW], f32, tag="sps", bufs=3, name="sps")[:, :wdt]
                for n_cc, cc in enumerate(cc_early):
                    nc.tensor.matmul(
                        ps, ctx_all[:, cc, tsl], wop_sb[:, cc, off:off + wdt],
                        start=(n_cc == 0), stop=(n_cc == len(cc_early) - 1),
                    )
                nc.vector.tensor_tensor(
                    y_sb[:, c, osl], ps, reps["bo"][:, osl], Alu.add
                )
                nc.vector.tensor_tensor(
                    y_sb[:, c, osl], y_sb[:, c, osl], xo[:, c, osl], Alu.add
                )
        for c in range(TC):
            tsl = slice(c * P, (c + 1) * P)
            for off, wdt in ((0, 512), (512, 256)):
                osl = slice(off, off + wdt)
                ps = att_ps.tile([P, QW], f32, tag="sps", bufs=3, name="sps")[:, :wdt]
                for n_cc, cc in enumerate(cc_late):
                    nc.tensor.matmul(
                        ps, ctx_all[:, cc, tsl], wop_sb[:, cc, off:off + wdt],
                        start=(n_cc == 0), stop=(n_cc == len(cc_late) - 1),
                    )
                nc.vector.tensor_tensor(
                    y_sb[:, c, osl], y_sb[:, c, osl], ps, Alu.add
                )
        att_ps.release()
        ph3_ps = tc.alloc_tile_pool(name="ph3_ps", bufs=1, space="PSUM")
        att_sb.release()

        if DBG:
            nc.sync.dma_start(dbg_y[:, :], y_sb.rearrange("p a b -> p (a b)"))
        # ================ phase 4: LN2 + transpose ================
        ffn_sb = tc.alloc_tile_pool(name="ffn_sb", bufs=1)
        stats2 = ffn_sb.tile([P, TC, 4], f32)
        s2 = stats2[:, :, 0]
        ss2 = stats2[:, :, 1]
        m2 = stats2[:, :, 2]
        r2 = stats2[:, :, 3]
        y2 = ffn_sb.tile([P, TC, E], bf16)
        y2T = ffn_sb.tile([P, EC, T], bf16)
        var2 = ffn_sb.tile([P, TC], f32)
        msq2 = ffn_sb.tile([P, TC], f32)
        lnv2 = ffn_sb.tile([P, TC], f32)
        for c in range(TC):
            sq2 = stream.tile([P, E], f32, tag="sq2", bufs=2, name="sq2")
            nc.vector.tensor_reduce(s2[:, c, None], y_sb[:, c, :], Axis.X, Alu.add)
            nc.scalar.activation(sq2, y_sb[:, c, :], Act.Square)
            nc.vector.tensor_reduce(ss2[:, c, None], sq2, Axis.X, Alu.add)
            nc.vector.tensor_scalar(m2[:, c, None], s2[:, c, None], 1.0 / E, None, Alu.mult)
            nc.vector.tensor_scalar(var2[:, c, None], ss2[:, c, None], 1.0 / E, None, Alu.mult)
            nc.vector.tensor_tensor(msq2[:, c, None], m2[:, c, None], m2[:, c, None], Alu.mult)
            nc.vector.tensor_tensor(var2[:, c, None], var2[:, c, None], msq2[:, c, None], Alu.subtract)
            nc.scalar.activation(lnv2[:, c, None], var2[:, c, None], Act.Ln, bias=eps_col2)
            nc.scalar.activation(r2[:, c, None], lnv2[:, c, None], Act.Exp, scale=-0.5)
            nc.vector.tensor_scalar(
                y2[:, c, :], y_sb[:, c, :],
                m2[:, c, None], r2[:, c, None],
                Alu.subtract, Alu.mult,
            )
            nc.vector.tensor_tensor(y2[:, c, :], y2[:, c, :], reps["ln2_g"], Alu.mult)
            nc.vector.tensor_tensor(y2[:, c, :], y2[:, c, :], reps["ln2_b"], Alu.add)
            for ec in range(EC):
                tps = ph3_ps.tile([P, P], bf16, tag="tp", bufs=2, name="tp")
                nc.tensor.transpose(tps, y2[:, c, ec * P:(ec + 1) * P], ident)
                nc.vector.tensor_copy(y2T[:, ec, c * P:(c + 1) * P], tps)
        ph3_ps.release()

        if DBG:
            nc.sync.dma_start(dbg_y2t[:, :], y2T.rearrange("p a b -> p (a b)"))
        # ================ phase 5: FFN ================
        ffn_ps = tc.alloc_tile_pool(name="ffn_ps", bufs=1, space="PSUM")
        hT = ffn_sb.tile([P, FC, T], bf16)
        w2_sb = ffn_sb.tile([P, FC, E], bf16)
        nc.sync.dma_start(w2_sb, w2_in.rearrange("(c p) o -> p c o", p=P))
        for fc in range(FC):
            w1b = ffn_sb.tile([P, EC, P], bf16, tag="w1b", bufs=3, name="w1b")
            nc.sync.dma_start(
                w1b, w1_in[:, fc * P:(fc + 1) * P].rearrange("(c p) h -> p c h", p=P)
            )
            hps = ffn_ps.tile([P, T], f32, tag="h", bufs=3, name="h")
            for ec in range(EC):
                nc.tensor.matmul(
                    hps, w1b[:, ec, :], y2T[:, ec, :],
                    start=(ec == 0), stop=(ec == EC - 1),
                )
            nc.scalar.activation(hT[:, fc, :], hps, Act.Gelu, bias=b1_col[:, fc, None])

        if DBG:
            nc.sync.dma_start(dbg_h[:, :], hT.rearrange("p a b -> p (a b)"))
        for c in range(TC):
            tsl = slice(c * P, (c + 1) * P)
            za = ffn_ps.tile([P, 512], f32, tag="zf1", bufs=2, name="zf1")
            zb = ffn_ps.tile([P, 256], f32, tag="zf2", bufs=2, name="zf2")
            for fc in range(FC):
                nc.tensor.matmul(
                    za, hT[:, fc, tsl], w2_sb[:, fc, 0:512],
                    start=(fc == 0), stop=(fc == FC - 1),
                )
                nc.tensor.matmul(
                    zb, hT[:, fc, tsl], w2_sb[:, fc, 512:768],
                    start=(fc == 0), stop=(fc == FC - 1),
                )
            o_sb = stream.tile([P, E], f32, tag="o", bufs=2, name="o")
            nc.vector.tensor_tensor(o_sb[:, 0:512], za, y_sb[:, c, 0:512], Alu.add)
            nc.vector.tensor_tensor(o_sb[:, 512:768], zb, y_sb[:, c, 512:768], Alu.add)
            nc.vector.tensor_tensor(o_sb, o_sb, reps["b2"], Alu.add)
            nc.sync.dma_start(out_dram[c * P:(c + 1) * P, :], o_sb)

        ffn_ps.release()
        ffn_sb.release()
        stream.release()
        acts.release()
        const_pool.release()

    nc.finalize()
    return nc


def _get_nc():
    if "nc" not in _CACHE:
        _CACHE["nc"] = _build_nc()
    return _CACHE["nc"]


def _shard_inputs(inputs):
    import ml_dtypes

    bf16 = ml_dtypes.bfloat16
    x = np.asarray(inputs["x"], dtype=np.float32)
    f = {k: np.asarray(v, dtype=np.float32) for k, v in inputs.items() if k != "x"}

    xT = [np.ascontiguousarray(x[g].T).astype(bf16) for g in range(B)]
    wo = f["wo"]

    in_maps = []
    for c in range(NCORES):
        g, r = c // TPG, c % TPG
        hsl = slice(HD * r, HD * r + HD)

        wop = np.zeros((NCORES * HD, E), np.float32)
        for j in range(NCORES):
            if j // TPG == g:
                wop[j * HD:(j + 1) * HD] = wo[HD * (j % TPG): HD * (j % TPG) + HD]

        def pad(b):
            v = np.zeros(2 * P, np.float32)
            v[:HD] = b
            return v

        m = {
            "xT": xT[g],
            "x_own": np.ascontiguousarray(x[g, r * T:(r + 1) * T]),
            "wq": np.ascontiguousarray(f["wq"][:, hsl]).astype(bf16),
            "wk": np.ascontiguousarray(f["wk"][:, hsl]).astype(bf16),
            "wv": np.ascontiguousarray(f["wv"][:, hsl]).astype(bf16),
            "bq": pad(f["bq"][hsl]),
            "bk": pad(f["bk"][hsl]),
            "bv": np.ascontiguousarray(f["bv"][hsl]),
            "wop": wop.astype(bf16),
            "bo": f["bo"],
            "ln1_g": f["ln1_g"], "ln1_b": f["ln1_b"],
            "ln2_g": f["ln2_g"], "ln2_b": f["ln2_b"],
            "w1": f["w1"].astype(bf16), "b1": f["b1"],
            "w2": f["w2"].astype(bf16), "b2": f["b2"],
        }
        in_maps.append(m)
    return in_maps


def kernel(**inputs):
    from concourse.bass_utils import run_bass_kernel_spmd

    nc = _get_nc()
    in_maps = _shard_inputs(inputs)
    res = run_bass_kernel_spmd(nc, in_maps, core_ids=list(range(NCORES)))
    _CACHE["last_results"] = res
    out = np.empty((B, S, E), np.float32)
    for c in range(NCORES):
        g, r = c // TPG, c % TPG
        out[g, r * T:(r + 1) * T, :] = res.results[c]["out"]
    return out


# revision 28
# speedup vs baseline: 1.2592x; 1.0387x over previous
"""Trainium2 Bass kernel for a dense transformer block (B=2, S=2048, E=768, H=12).

Sharding: 8 cores = 2 batch groups x 4 ranks. Head-parallel attention:
core (g, r) owns heads [3r, 3r+3) of batch element g and token rows
[512r, 512r+512) for everything token-local (residuals, LN2, FFN, output).

The host replicates x^T (bf16) across each batch group, so LN1 stats and
Q/K/V projections for the core's own heads over the FULL sequence start
immediately with no collective. After attention, each core holds ctx for
its 3 heads over all 2048 tokens; a per-head 8-core AllToAll sends each
rank the ctx slice for its own 512 tokens. The receive frame interleaves
both batch groups; the output projection contracts over the full 1536-row
frame with a host-permuted wo whose cross-group rows are zeroed, keeping
the device program SPMD-uniform. FFN is token-parallel with full streamed
weights. All matmul operands are bf16 (fp32 PSUM accumulation); softmax
skips max-subtraction and gets the denominator via a ones-augmented V
column.
"""

import numpy as np

B, S, E, H, D = 2, 2048, 768, 12, 64
F = 4 * E
NCORES = 8
TPG = 4                 # ranks per batch group
T = S // TPG            # 512 own tokens
HPC = H // TPG          # 3 heads per core
HD = HPC * D            # 192 own head dims
P = 128
EC = E // P             # 6 embed chunks
FC = F // P             # 24 ffn-hidden chunks
TC = T // P             # 4 own token chunks
KC = S // P             # 16 key chunks (full seq)
QB = 2                  # query blocks of 1024
QW = S // QB            # 1024
EPS = 1e-5
SCALE = 1.0 / float(np.sqrt(E))

_CACHE = {}


def _build_nc():
    import concourse.bass as bass
    import concourse.mybir as mybir
    import concourse.tile as tile
    from concourse import bacc
    from concourse.masks import make_identity

    dt = mybir.dt
    f32 = dt.float32
    bf16 = dt.bfloat16
    Alu = mybir.AluOpType
    Act = mybir.ActivationFunctionType
    Axis = mybir.AxisListType

    nc = bacc.Bacc(
        "TRN2",
        target_bir_lowering=False,
        debug=False,
        enable_asserts=False,
        num_devices=NCORES,
    )

    xT_in = nc.dram_tensor("xT", [E, S], bf16, kind="ExternalInput")
    xo_in = nc.dram_tensor("x_own", [T, E], f32, kind="ExternalInput")
    wq_in = nc.dram_tensor("wq", [E, HD], bf16, kind="ExternalInput")
    wk_in = nc.dram_tensor("wk", [E, HD], bf16, kind="ExternalInput")
    wv_in = nc.dram_tensor("wv", [E, HD], bf16, kind="ExternalInput")
    bq_in = nc.dram_tensor("bq", [2 * P], f32, kind="ExternalInput")
    bk_in = nc.dram_tensor("bk", [2 * P], f32, kind="ExternalInput")
    bv_in = nc.dram_tensor("bv", [HD], f32, kind="ExternalInput")
    wop_in = nc.dram_tensor("wop", [NCORES * HD, E], bf16, kind="ExternalInput")
    bo_in = nc.dram_tensor("bo", [E], f32, kind="ExternalInput")
    ln1g_in = nc.dram_tensor("ln1_g", [E], f32, kind="ExternalInput")
    ln1b_in = nc.dram_tensor("ln1_b", [E], f32, kind="ExternalInput")
    ln2g_in = nc.dram_tensor("ln2_g", [E], f32, kind="ExternalInput")
    ln2b_in = nc.dram_tensor("ln2_b", [E], f32, kind="ExternalInput")
    w1_in = nc.dram_tensor("w1", [E, F], bf16, kind="ExternalInput")
    b1_in = nc.dram_tensor("b1", [F], f32, kind="ExternalInput")
    w2_in = nc.dram_tensor("w2", [F, E], bf16, kind="ExternalInput")
    b2_in = nc.dram_tensor("b2", [E], f32, kind="ExternalInput")
    out_dram = nc.dram_tensor("out", [T, E], f32, kind="ExternalOutput")
    import os as _os
    DBG = bool(_os.environ.get("KBUILD_DEBUG"))
    if DBG:
        dbg_rs = nc.dram_tensor("dbg_rs", [P, S], bf16, kind="ExternalOutput")
        dbg_mu = nc.dram_tensor("dbg_mu", [P, S], bf16, kind="ExternalOutput")
        dbg_xh = nc.dram_tensor("dbg_xh", [P, S], bf16, kind="ExternalOutput")
        dbg_kt = nc.dram_tensor("dbg_kt", [P, S], bf16, kind="ExternalOutput")
        dbg_v3 = nc.dram_tensor("dbg_v3", [P, KC * HPC * (D + 1)], bf16, kind="ExternalOutput")
        dbg_ctx = nc.dram_tensor("dbg_ctx", [HPC, 64, S], bf16, kind="ExternalOutput")
        dbg_a2i = nc.dram_tensor("dbg_a2i", [NCORES, D, T], bf16, kind="ExternalOutput")
        dbg_a2o = nc.dram_tensor("dbg_a2o", [NCORES, D, T], bf16, kind="ExternalOutput")
        dbg_ca = nc.dram_tensor("dbg_ca", [P, 2 * EC * T], bf16, kind="ExternalOutput")
        dbg_y = nc.dram_tensor("dbg_y", [P, TC * E], f32, kind="ExternalOutput")
        dbg_y2t = nc.dram_tensor("dbg_y2t", [P, EC * T], bf16, kind="ExternalOutput")
        dbg_h = nc.dram_tensor("dbg_h", [P, FC * T], bf16, kind="ExternalOutput")

    # per-own-head AllToAll bounce buffers
    a2a_in = nc.dram_tensor("a2a_in", [HPC, NCORES, D, T], bf16)
    a2a_out = nc.dram_tensor("a2a_out", [HPC, NCORES, D, T], bf16)
    a2a_groups = [list(range(NCORES))]

    with tile.TileContext(nc) as tc:
        const_pool = tc.alloc_tile_pool(name="const", bufs=1)
        acts = tc.alloc_tile_pool(name="acts", bufs=1)
        stream = tc.alloc_tile_pool(name="stream", bufs=1)

        # ---------------- constants ----------------
        ident = const_pool.tile([P, P], bf16)
        make_identity(nc, ident)
        ones_col = const_pool.tile([P, 1], bf16)
        nc.vector.memset(ones_col, 1.0)
        ones64 = const_pool.tile([1, 64], bf16)
        nc.vector.memset(ones64, 1.0)
        eps_col = const_pool.tile([1, 1], f32)
        nc.vector.memset(eps_col, EPS)
        eps_col2 = const_pool.tile([P, 1], f32)
        nc.vector.memset(eps_col2, EPS)

        ln1g_col = const_pool.tile([P, EC], f32)
        nc.sync.dma_start(ln1g_col, ln1g_in.rearrange("(c p) -> p c", p=P))
        ln1b_col = const_pool.tile([P, EC], f32)
        nc.sync.dma_start(ln1b_col, ln1b_in.rearrange("(c p) -> p c", p=P))
        bqc = const_pool.tile([P, 2], f32)
        nc.sync.dma_start(bqc, bq_in.rearrange("(c p) -> p c", p=P))
        bkc = const_pool.tile([P, 2], f32)
        nc.sync.dma_start(bkc, bk_in.rearrange("(c p) -> p c", p=P))
        b1_col = const_pool.tile([P, FC], f32)
        nc.sync.dma_start(b1_col, b1_in.rearrange("(c p) -> p c", p=P))

        # free-axis rows replicated across partitions
        reps = {}
        for name, t_in, width in [
            ("bv", bv_in, HD), ("bo", bo_in, E), ("b2", b2_in, E),
            ("ln2_g", ln2g_in, E), ("ln2_b", ln2b_in, E),
        ]:
            row = const_pool.tile([1, width], f32, name=f"{name}_row")
            nc.sync.dma_start(row, t_in[None, :])
            rep = const_pool.tile([P, width], f32, name=f"{name}_rep")
            nc.gpsimd.partition_broadcast(rep, row)
            reps[name] = rep

        # ================ phase 1: stats, x-hat, QKV ================
        ph1_sb = tc.alloc_tile_pool(name="ph1_sb", bufs=1)
        ph1a_ps = tc.alloc_tile_pool(name="ph1a_ps", bufs=1, space="PSUM")

        xt = ph1_sb.tile([P, EC, S], bf16)
        xt_v = xT_in.rearrange("(c p) t -> p c t", p=P)
        for ec in range(EC):
            nc.sync.dma_start(xt[:, ec, :], xt_v[:, ec, :])
        wq_sb = ph1_sb.tile([P, EC, HD], bf16)
        nc.sync.dma_start(wq_sb, wq_in.rearrange("(c p) d -> p c d", p=P))
        wk_sb = ph1_sb.tile([P, EC, HD], bf16)
        nc.sync.dma_start(wk_sb, wk_in.rearrange("(c p) d -> p c d", p=P))
        wv_sb = ph1_sb.tile([P, EC, HD], bf16)
        nc.sync.dma_start(wv_sb, wv_in.rearrange("(c p) d -> p c d", p=P))
        xo = acts.tile([P, TC, E], f32)
        nc.sync.dma_start(xo, xo_in.rearrange("(c p) e -> p c e", p=P))

        # LN1 stats for all 2048 tokens: col-sums of x and x^2 via PE
        st_s = [
            ph1a_ps.tile([1, 512], f32, tag=f"sts{qb}", bufs=1, name=f"sts{qb}")
            for qb in range(4)
        ]
        st_q = [
            ph1a_ps.tile([1, 512], f32, tag=f"stq{qb}", bufs=1, name=f"stq{qb}")
            for qb in range(4)
        ]
        for ec in range(EC):
            for qb in range(4):
                sl = slice(qb * 512, (qb + 1) * 512)
                nc.tensor.matmul(
                    st_s[qb], ones_col, xt[:, ec, sl],
                    start=(ec == 0), stop=(ec == EC - 1),
                )
        for ec in range(EC):
            sq = stream.tile([P, S], bf16, tag="sq", bufs=2, name="sq")
            nc.scalar.activation(sq, xt[:, ec, :], Act.Square)
            for qb in range(4):
                sl = slice(qb * 512, (qb + 1) * 512)
                nc.tensor.matmul(
                    st_q[qb], ones_col, sq[:, sl],
                    start=(ec == 0), stop=(ec == EC - 1),
                )

        rs_b = ph1_sb.tile([P, S], bf16)
        murs_b = ph1_sb.tile([P, S], bf16)
        for qb in range(4):
            sl = slice(qb * 512, (qb + 1) * 512)
            mean = ph1_sb.tile([1, 512], f32, name=f"mean{qb}")
            nc.vector.tensor_scalar(mean, st_s[qb], 1.0 / E, None, Alu.mult)
            var = ph1_sb.tile([1, 512], f32, name=f"var{qb}")
            nc.vector.tensor_scalar(var, st_q[qb], 1.0 / E, None, Alu.mult)
            msq = ph1_sb.tile([1, 512], f32, name=f"msq{qb}")
            nc.vector.tensor_tensor(msq, mean, mean, Alu.mult)
            nc.vector.tensor_tensor(var, var, msq, Alu.subtract)
            lnv = ph1_sb.tile([1, 512], f32, name=f"lnv{qb}")
            nc.scalar.activation(lnv, var, Act.Ln, bias=eps_col)
            rsq = ph1_sb.tile([1, 512], f32, name=f"rsq{qb}")
            nc.scalar.activation(rsq, lnv, Act.Exp, scale=-0.5)
            rs_bf = ph1_sb.tile([1, 512], bf16, name=f"rsbf{qb}")
            nc.vector.tensor_copy(rs_bf, rsq)
            murs_bf = ph1_sb.tile([1, 512], bf16, name=f"mursbf{qb}")
            nc.vector.tensor_tensor(murs_bf, mean, rsq, Alu.mult)
            nc.gpsimd.partition_broadcast(rs_b[:, sl], rs_bf)
            nc.gpsimd.partition_broadcast(murs_b[:, sl], murs_bf)

        # x-hat^T = ((x*rs) - mu*rs) * g + b   (bf16, in-place over xt)
        xhat = xt
        for ec in range(EC):
            t1 = stream.tile([P, S], bf16, tag="xh1", bufs=2, name="xh1")
            nc.vector.tensor_tensor(t1, xt[:, ec, :], rs_b, Alu.mult)
            nc.vector.tensor_tensor(t1, t1, murs_b, Alu.subtract)
            nc.vector.tensor_scalar(
                xhat[:, ec, :], t1,
                ln1g_col[:, ec, None], ln1b_col[:, ec, None],
                Alu.mult, Alu.add,
            )

        ph1a_ps.release()
        ph1b_ps = tc.alloc_tile_pool(name="ph1b_ps", bufs=1, space="PSUM")

        # K^T and Q^T for own heads over all tokens: [HD rows, S]
        kT_a = acts.tile([P, S], bf16)
        kT_b = acts.tile([64, S], bf16)
        qT_a = acts.tile([P, S], bf16)
        qT_b = acts.tile([64, S], bf16)
        for (w_sb, bc_col, dst_a, dst_b) in (
            (wk_sb, bkc, kT_a, kT_b),
            (wq_sb, bqc, qT_a, qT_b),
        ):
            for qb in range(4):
                sl = slice(qb * 512, (qb + 1) * 512)
                psa = ph1b_ps.tile([P, 512], f32, tag="proj", bufs=2, name="proj")
                psb = ph1b_ps.tile([64, 512], f32, tag="projB", bufs=2, name="projB")
                for ec in range(EC):
                    nc.tensor.matmul(
                        psa, w_sb[:, ec, 0:P], xhat[:, ec, sl],
                        start=(ec == 0), stop=(ec == EC - 1),
                    )
                    nc.tensor.matmul(
                        psb, w_sb[:, ec, P:HD], xhat[:, ec, sl],
                        start=(ec == 0), stop=(ec == EC - 1),
                    )
                nc.vector.tensor_scalar(
                    dst_a[:, sl], psa, bc_col[:, 0, None], None, Alu.add
                )
                nc.vector.tensor_scalar(
                    dst_b[:, sl], psb, bc_col[0:64, 1, None], None, Alu.add
                )

        # V natural (per key chunk), ones-augmented: [128k, KC, HPC, D+1]
        v3 = acts.tile([P, KC, HPC, D + 1], bf16)
        for kc in range(KC):
            for i in range(HPC):
                nc.vector.memset(v3[:, kc, i, D, None], 1.0)
        for kc in range(KC):
            vp = ph1b_ps.tile([P, HD], f32, tag="vp", bufs=2, name="vp")
            tsl = slice(kc * P, (kc + 1) * P)
            for ec in range(EC):
                nc.tensor.matmul(
                    vp, xhat[:, ec, tsl], wv_sb[:, ec, :],
                    start=(ec == 0), stop=(ec == EC - 1),
                )
            for i in range(HPC):
                nc.vector.tensor_tensor(
                    v3[:, kc, i, 0:D], vp[:, i * D:(i + 1) * D],
                    reps["bv"][:, i * D:(i + 1) * D], Alu.add,
                )

        if DBG:
            nc.sync.dma_start(dbg_rs[:, :], rs_b)
            nc.sync.dma_start(dbg_mu[:, :], murs_b)
            nc.sync.dma_start(dbg_xh[:, :], xhat[:, 0, :])
            nc.sync.dma_start(dbg_kt[:, :], kT_a)
            nc.sync.dma_start(dbg_v3[:, :], v3.rearrange("p a b c -> p (a b c)"))

        ph1_sb.release()
        ph1b_ps.release()

        # ================ phase 2: attention (3 own heads) ================
        att_sb = tc.alloc_tile_pool(name="att_sb", bufs=1)
        att_ps = tc.alloc_tile_pool(name="att_ps", bufs=1, space="PSUM")

        # prefetch heavy phase-3 weights early (overlaps attention)
        wop_sb = att_sb.tile([P, 2 * EC, E], bf16)
        nc.sync.dma_start(wop_sb, wop_in.rearrange("(c p) o -> p c o", p=P))

        for i in range(HPC):
            if i == 0:
                krows, qrows = kT_a[0:64], qT_a[0:64]
            elif i == 1:
                krows, qrows = kT_a[64:128], qT_a[64:128]
            else:
                krows, qrows = kT_b[0:64], qT_b[0:64]
            ctxT = att_sb.tile([64, S], bf16, tag="ctxT", bufs=2, name="ctxT")
            for qb in range(QB):
                ctx_ps = att_ps.tile([D + 1, QW], f32, tag="ctx", bufs=1, name="ctx")
                exps = [None] * KC

                def emit_av(kc):
                    for h2 in range(2):
                        nc.tensor.matmul(
                            ctx_ps[:, h2 * 512:(h2 + 1) * 512],
                            v3[:, kc, i, :],
                            exps[kc][:, h2 * 512:(h2 + 1) * 512],
                            start=(kc == 0), stop=(kc == KC - 1),
                        )

                for kc in range(KC):
                    s_ps = att_ps.tile([P, QW], f32, tag="sps", bufs=3, name="sps")
                    for h2 in range(2):
                        nc.tensor.matmul(
                            s_ps[:, h2 * 512:(h2 + 1) * 512],
                            krows[:, kc * P:(kc + 1) * P],
                            qrows[:, qb * QW + h2 * 512: qb * QW + (h2 + 1) * 512],
                            start=True, stop=True,
                        )
                    exps[kc] = att_sb.tile([P, QW], bf16, tag="exp", bufs=4, name="exp")
                    nc.scalar.activation(exps[kc], s_ps, Act.Exp, scale=SCALE)
                    if kc >= 1:
                        emit_av(kc - 1)
                emit_av(KC - 1)
                den = att_sb.tile([1, QW], f32, tag="den", bufs=2, name="den")
                nc.vector.tensor_copy(den, ctx_ps[D:D + 1, :])
                den_f = att_sb.tile([1, QW], f32, tag="denf", bufs=2, name="denf")
                nc.vector.reciprocal_approx_fast(den_f, den)
                bc_rep = att_sb.tile([64, QW], f32, tag="bcr", bufs=2, name="bcr")
                nc.gpsimd.partition_broadcast(bc_rep, den_f)
                nc.vector.tensor_tensor(
                    ctxT[:, qb * QW:(qb + 1) * QW], ctx_ps[0:64, :], bc_rep, Alu.mult
                )
            if DBG:
                nc.sync.dma_start(dbg_ctx[i], ctxT)
            for j in range(NCORES):
                r = j % TPG
                nc.sync.dma_start(a2a_in[i, j], ctxT[:, r * T:(r + 1) * T])
            nc.gpsimd.collective_compute(
                "AllToAll", mybir.AluOpType.bypass,
                replica_groups=a2a_groups,
                ins=[a2a_in[i]],
                outs=[a2a_out[i]],
            )

        # ================ phase 3: assemble ctx, out-proj, residual =======
        # frame row j*HD + i*D + d  <->  a2a_out[i, j, d, :]
        ctx_all = att_sb.tile([P, 2 * EC, T], bf16)
        for i in range(HPC):
            for j in range(NCORES):
                row = j * HD + i * D
                cc, po = row // P, row % P
                nc.sync.dma_start(ctx_all[po:po + D, cc, :], a2a_out[i, j])

        if DBG:
            nc.sync.dma_start(dbg_ca[:, :], ctx_all.rearrange("p a b -> p (a b)"))
            for j in range(NCORES):
                bnc_i = stream.tile([64, T], bf16, tag="bnci", bufs=1, name="bnci")
                nc.sync.dma_start(bnc_i, a2a_in[0, j])
                nc.sync.dma_start(dbg_a2i[j], bnc_i)
                bnc_o = stream.tile([64, T], bf16, tag="bnco", bufs=1, name="bnco")
                nc.sync.dma_start(bnc_o, a2a_out[0, j])
                nc.sync.dma_start(dbg_a2o[j], bnc_o)
        y_sb = acts.tile([P, TC, E], f32)
        cc_early = [0, 3, 6, 9]
        cc_late = [1, 2, 4, 5, 7, 8, 10, 11]
        for c in range(TC):
            tsl = slice(c * P, (c + 1) * P)
            for off, wdt in ((0, 512), (512, 256)):
                osl = slice(off, off + wdt)
                ps = att_ps.tile([P, QW], f32, tag="sps", bufs=3, name="sps")[:, :wdt]
                for n_cc, cc in enumerate(cc_early):
                    nc.tensor.matmul(
                        ps, ctx_all[:, cc, tsl], wop_sb[:, cc, off:off + wdt],
                        start=(n_cc == 0), stop=(n_cc == len(cc_early) - 1),
                    )
                nc.vector.tensor_tensor(
                    y_sb[:, c, osl], ps, reps["bo"][:, osl], Alu.add
                )
                nc.vector.tensor_tensor(
                    y_sb[:, c, osl], y_sb[:, c, osl], xo[:, c, osl], Alu.add
                )
        for c in range(TC):
            tsl = slice(c * P, (c + 1) * P)
            for off, wdt in ((0, 512), (512, 256)):
                osl = slice(off, off + wdt)
                ps = att_ps.tile([P, QW], f32, tag="sps", bufs=3, name="sps")[:, :wdt]
                for n_cc, cc in enumerate(cc_late):
                    nc.tensor.matmul(
                        ps, ctx_all[:, cc, tsl], wop_sb[:, cc, off:off + wdt],
                        start=(n_cc == 0), stop=(n_cc == len(cc_late) - 1),
                    )
                nc.vector.tensor_tensor(
                    y_sb[:, c, osl], y_sb[:, c, osl], ps, Alu.add
                )
        att_ps.release()
        ph3_ps = tc.alloc_tile_pool(name="ph3_ps", bufs=1, space="PSUM")
        att_sb.release()

        if DBG:
            nc.sync.dma_start(dbg_y[:, :], y_sb.rearrange("p a b -> p (a b)"))
        # ================ phase 4: LN2 + transpose ================
        ffn_sb = tc.alloc_tile_pool(name="ffn_sb", bufs=1)
        stats2 = ffn_sb.tile([P, TC, 4], f32)
        s2 = stats2[:, :, 0]
        ss2 = stats2[:, :, 1]
        m2 = stats2[:, :, 2]
        r2 = stats2[:, :, 3]
        y2 = ffn_sb.tile([P, TC, E], bf16)
        y2T = ffn_sb.tile([P, EC, T], bf16)
        var2 = ffn_sb.tile([P, TC], f32)
        msq2 = ffn_sb.tile([P, TC], f32)
        lnv2 = ffn_sb.tile([P, TC], f32)
        for c in range(TC):
            sq2 = stream.tile([P, E], f32, tag="sq2", bufs=2, name="sq2")
            nc.vector.tensor_reduce(s2[:, c, None], y_sb[:, c, :], Axis.X, Alu.add)
            nc.scalar.activation(sq2, y_sb[:, c, :], Act.Square)
            nc.vector.tensor_reduce(ss2[:, c, None], sq2, Axis.X, Alu.add)
            nc.vector.tensor_scalar(m2[:, c, None], s2[:, c, None], 1.0 / E, None, Alu.mult)
            nc.vector.tensor_scalar(var2[:, c, None], ss2[:, c, None], 1.0 / E, None, Alu.mult)
            nc.vector.tensor_tensor(msq2[:, c, None], m2[:, c, None], m2[:, c, None], Alu.mult)
            nc.vector.tensor_tensor(var2[:, c, None], var2[:, c, None], msq2[:, c, None], Alu.subtract)
            nc.scalar.activation(lnv2[:, c, None], var2[:, c, None], Act.Ln, bias=eps_col2)
            nc.scalar.activation(r2[:, c, None], lnv2[:, c, None], Act.Exp, scale=-0.5)
            nc.vector.tensor_scalar(
                y2[:, c, :], y_sb[:, c, :],
                m2[:, c, None], r2[:, c, None],
                Alu.subtract, Alu.mult,
            )
            nc.vector.tensor_tensor(y2[:, c, :], y2[:, c, :], reps["ln2_g"], Alu.mult)
            nc.vector.tensor_tensor(y2[:, c, :], y2[:, c, :], reps["ln2_b"], Alu.add)
            for ec in range(EC):
                tps = ph3_ps.tile([P, P], bf16, tag="tp", bufs=2, name="tp")
                nc.tensor.transpose(tps, y2[:, c, ec * P:(ec + 1) * P], ident)
                nc.vector.tensor_copy(y2T[:, ec, c * P:(c + 1) * P], tps)
        ph3_ps.release()

        if DBG:
            nc.sync.dma_start(dbg_y2t[:, :], y2T.rearrange("p a b -> p (a b)"))
        # ================ phase 5: FFN ================
        ffn_ps = tc.alloc_tile_pool(name="ffn_ps", bufs=1, space="PSUM")
        hT = ffn_sb.tile([P, FC, T], bf16)
        w2_sb = ffn_sb.tile([P, FC, E], bf16)
        nc.sync.dma_start(w2_sb, w2_in.rearrange("(c p) o -> p c o", p=P))
        for fc in range(FC):
            w1b = ffn_sb.tile([P, EC, P], bf16, tag="w1b", bufs=3, name="w1b")
            nc.sync.dma_start(
                w1b, w1_in[:, fc * P:(fc + 1) * P].rearrange("(c p) h -> p c h", p=P)
            )
            hps = ffn_ps.tile([P, T], f32, tag="h", bufs=3, name="h")
            for ec in range(EC):
                nc.tensor.matmul(
                    hps, w1b[:, ec, :], y2T[:, ec, :],
                    start=(ec == 0), stop=(ec == EC - 1),
                )
            nc.scalar.activation(hT[:, fc, :], hps, Act.Gelu, bias=b1_col[:, fc, None])

        if DBG:
            nc.sync.dma_start(dbg_h[:, :], hT.rearrange("p a b -> p (a b)"))
        for c in range(TC):
            tsl = slice(c * P, (c + 1) * P)
            za = ffn_ps.tile([P, 512], f32, tag="zf1", bufs=2, name="zf1")
            zb = ffn_ps.tile([P, 256], f32, tag="zf2", bufs=2, name="zf2")
            for fc in range(FC):
                nc.tensor.matmul(
                    za, hT[:, fc, tsl], w2_sb[:, fc, 0:512],
                    start=(fc == 0), stop=(fc == FC - 1),
                )
                nc.tensor.matmul(
                    zb, hT[:, fc, tsl], w2_sb[:, fc, 512:768],
                    start=(fc == 0), stop=(fc == FC - 1),
                )
            o_sb = stream.tile([P, E], f32, tag="o", bufs=2, name="o")
            nc.vector.tensor_tensor(o_sb[:, 0:512], za, y_sb[:, c, 0:512], Alu.add)
            nc.vector.tensor_tensor(o_sb[:, 512:768], zb, y_sb[:, c, 512:768], Alu.add)
            nc.vector.tensor_tensor(o_sb, o_sb, reps["b2"], Alu.add)
            nc.sync.dma_start(out_dram[c * P:(c + 1) * P, :], o_sb)

        ffn_ps.release()
        ffn_sb.release()
        stream.release()
        acts.release()
        const_pool.release()

    nc.finalize()
    return nc


def _get_nc():
    if "nc" not in _CACHE:
        _CACHE["nc"] = _build_nc()
    return _CACHE["nc"]


def _shard_inputs(inputs):
    import ml_dtypes

    bf16 = ml_dtypes.bfloat16
    x = np.asarray(inputs["x"], dtype=np.float32)
    f = {k: np.asarray(v, dtype=np.float32) for k, v in inputs.items() if k != "x"}

    xT = [np.ascontiguousarray(x[g].T).astype(bf16) for g in range(B)]
    wo = f["wo"]

    in_maps = []
    for c in range(NCORES):
        g, r = c // TPG, c % TPG
        hsl = slice(HD * r, HD * r + HD)

        wop = np.zeros((NCORES * HD, E), np.float32)
        for j in range(NCORES):
            if j // TPG == g:
                wop[j * HD:(j + 1) * HD] = wo[HD * (j % TPG): HD * (j % TPG) + HD]

        def pad(b):
            v = np.zeros(2 * P, np.float32)
            v[:HD] = b
            return v

        m = {
            "xT": xT[g],
            "x_own": np.ascontiguousarray(x[g, r * T:(r + 1) * T]),
            "wq": np.ascontiguousarray(f["wq"][:, hsl]).astype(bf16),
            "wk": np.ascontiguousarray(f["wk"][:, hsl]).astype(bf16),
            "wv": np.ascontiguousarray(f["wv"][:, hsl]).astype(bf16),
            "bq": pad(f["bq"][hsl]),
            "bk": pad(f["bk"][hsl]),
            "bv": np.ascontiguousarray(f["bv"][hsl]),
            "wop": wop.astype(bf16),
            "bo": f["bo"],
            "ln1_g": f["ln1_g"], "ln1_b": f["ln1_b"],
            "ln2_g": f["ln2_g"], "ln2_b": f["ln2_b"],
            "w1": f["w1"].astype(bf16), "b1": f["b1"],
            "w2": f["w2"].astype(bf16), "b2": f["b2"],
        }
        in_maps.append(m)
    return in_maps


def kernel(**inputs):
    from concourse.bass_utils import run_bass_kernel_spmd

    nc = _get_nc()
    in_maps = _shard_inputs(inputs)
    res = run_bass_kernel_spmd(nc, in_maps, core_ids=list(range(NCORES)))
    _CACHE["last_results"] = res
    out = np.empty((B, S, E), np.float32)
    for c in range(NCORES):
        g, r = c // TPG, c % TPG
        out[g, r * T:(r + 1) * T, :] = res.results[c]["out"]
    return out
